# revision 23
# baseline (speedup 1.0000x reference)
"""Trainium2 Bass kernel for a full MHA transformer block.

Reference computation (per batch element, data-parallel over 8 NeuronCores):
    qh/kh/vh = (x @ W + b) split into 16 heads of 64
    attn     = softmax(qh @ kh^T / sqrt(64))
    ctx      = attn @ vh
    out      = LayerNorm(gelu(ctx @ Wo + bo) + residual) * gamma + beta

Shapes: B=8, S=1024, D=1024, H=16, DH=64, fp32.

Layout strategy (per core):
  - Inputs are transposed on-chip (PE transpose) to X^T [d, s] so every GEMM
    keeps its contraction dim on SBUF partitions.
  - Q, K are produced transposed (Q^T/K^T: [feature, s]); V natural [s, feature].
  - Scores are computed transposed (scores^T [k, q]) two heads at a time using
    PE row-tiling (each head only uses 64 contraction rows).
  - exp(x/8) on ScalarE straight out of PSUM; no max-subtraction (scores are
    small by construction, fp32 exp is safe).
  - ctx^T [d, q] = V_ext^T-style matmul with an appended ones column producing
    the softmax denominator for free; normalization via a broadcast reciprocal.
  - ctx^T feeds the output projection as the stationary operand, producing the
    output in natural [s, d] layout for gelu/residual/LayerNorm.
"""

import numpy as np

S, D, H, DH = 1024, 1024, 16, 64
EPS = 1e-5
NCORES = 8
P = 128
SC = S // P    # seq chunks (8)
DC = D // P    # feature chunks (8)
HP = H // 2    # head pairs (8)

_cache = {}


def _build(flags, debug=False):
    from contextlib import ExitStack

    import concourse.bass as bass
    import concourse.mybir as mybir
    import concourse.tile as tile
    from concourse import bacc
    from concourse.masks import make_identity

    f32 = mybir.dt.float32
    f32r = mybir.dt.float32r
    bf16 = mybir.dt.bfloat16
    AF = mybir.ActivationFunctionType
    Alu = mybir.AluOpType

    use_bq, use_bk, use_bv, use_bo, use_gam, use_bet = flags

    nc = bacc.Bacc(None, target_bir_lowering=False)

    xq = nc.dram_tensor("xq", [S, D], f32, kind="ExternalInput")
    xk = nc.dram_tensor("xk", [S, D], f32, kind="ExternalInput")
    xv = nc.dram_tensor("xv", [S, D], f32, kind="ExternalInput")
    wq = nc.dram_tensor("wq", [D, D], f32, kind="ExternalInput")
    wk = nc.dram_tensor("wk", [D, D], f32, kind="ExternalInput")
    wv = nc.dram_tensor("wv", [D, D], f32, kind="ExternalInput")
    wo = nc.dram_tensor("wo", [D, D], f32, kind="ExternalInput")
    bq = nc.dram_tensor("bq", [D], f32, kind="ExternalInput")
    bk = nc.dram_tensor("bk", [D], f32, kind="ExternalInput")
    bv = nc.dram_tensor("bv", [D], f32, kind="ExternalInput")
    bo = nc.dram_tensor("bo", [D], f32, kind="ExternalInput")
    gam = nc.dram_tensor("gam", [D], f32, kind="ExternalInput")
    bet = nc.dram_tensor("bet", [D], f32, kind="ExternalInput")
    out = nc.dram_tensor("out", [S, D], f32, kind="ExternalOutput")
    if debug:
        d_qt = nc.dram_tensor("d_qt", [P, DC, S], f32, kind="ExternalOutput")
        d_kt = nc.dram_tensor("d_kt", [P, DC, S], f32, kind="ExternalOutput")
        d_vx = nc.dram_tensor("d_vx", [P, SC, H, DH + 1], f32, kind="ExternalOutput")
        d_es = nc.dram_tensor("d_es", [P, 1024], f32, kind="ExternalOutput")
        d_pc = nc.dram_tensor("d_pc", [DH + 1, 512], f32, kind="ExternalOutput")
        d_ct = nc.dram_tensor("d_ct", [P, DC, S], f32, kind="ExternalOutput")

    def r32(ap):
        return ap.bitcast(f32r)

    with tile.TileContext(nc) as tc, ExitStack() as top:
        consts = top.enter_context(tc.tile_pool(name="consts", bufs=1))
        bigp = top.enter_context(tc.tile_pool(name="bigp", bufs=1))
        wp = top.enter_context(tc.tile_pool(name="wp", bufs=1))

        ident = consts.tile([P, P], f32, tag="ident")
        make_identity(nc, ident[:])

        need_ones = use_bv or use_bo
        if need_ones:
            ones1 = consts.tile([1, P], f32r, tag="ones1")
            nc.vector.memset(ones1[:], 1.0)
        if use_bq:
            bq_sb = consts.tile([P, DC], f32, tag="bq")
            nc.sync.dma_start(out=bq_sb[:], in_=bq[:].rearrange("(c p) -> p c", p=P))
        if use_bk:
            bk_sb = consts.tile([P, DC], f32, tag="bk")
            nc.sync.dma_start(out=bk_sb[:], in_=bk[:].rearrange("(c p) -> p c", p=P))
        if use_bv:
            bv_sb = consts.tile([1, D], f32r, tag="bv")
            nc.sync.dma_start(out=bv_sb[:], in_=bv[:].rearrange("d -> 1 d").bitcast(f32r))
        if use_bo:
            bo_sb = consts.tile([1, D], f32r, tag="bo")
            nc.sync.dma_start(out=bo_sb[:], in_=bo[:].rearrange("d -> 1 d").bitcast(f32r))
        if use_gam:
            gam_bc = consts.tile([P, D], f32, tag="gam")
            nc.sync.dma_start(
                out=gam_bc[:],
                in_=bass.AP(tensor=gam[:].tensor, offset=0, ap=[[0, P], [1, D]]),
            )
        if use_bet:
            bet_bc = consts.tile([P, D], f32, tag="bet")
            nc.sync.dma_start(
                out=bet_bc[:],
                in_=bass.AP(tensor=bet[:].tensor, offset=0, ap=[[0, P], [1, D]]),
            )
        eps_sb = consts.tile([P, 1], f32, tag="eps")
        nc.vector.memset(eps_sb[:], EPS)

        def load_w(wd):
            w_sb = wp.tile([P, DC, D], f32r, tag="w")
            for kc in range(DC):
                nc.sync.dma_start(out=w_sb[:, kc, :], in_=wd[kc * P:(kc + 1) * P, :].bitcast(f32r))
            return w_sb

        with tc.tile_pool(name="qkvp", bufs=1) as qkvp:
            qt = qkvp.tile([P, DC, S], f32r, tag="qt")
            kt = qkvp.tile([P, DC, S], f32r, tag="kt")
            vx = qkvp.tile([P, SC, H, DH + 1], bf16, tag="vx")
            ones16 = consts.tile([P, H], f32, tag="ones16")
            nc.vector.memset(ones16[:], 1.0)
            for sc in range(SC):
                nc.vector.tensor_copy(vx[:, sc, :, DH], ones16[:])

            # ---------------- Phase A: transposes + projections ----------------
            with tc.tile_pool(name="xnp", bufs=3) as xnp, \
                 tc.tile_pool(name="tp_ps", bufs=2, space="PSUM") as tp_ps, \
                 tc.tile_pool(name="pj_ps", bufs=4, space="PSUM") as pj_ps:

                def transpose_in(xd):
                    xt = bigp.tile([P, DC, S], f32r, tag="big")
                    for sc in range(SC):
                        xn = xnp.tile([P, D], f32, tag="xn")
                        nc.sync.dma_start(out=xn[:], in_=xd[sc * P:(sc + 1) * P, :])
                        for kc in range(DC):
                            pt = tp_ps.tile([P, P], f32, tag="tp")
                            nc.tensor.transpose(
                                pt[:], xn[:, kc * P:(kc + 1) * P], ident[:]
                            )
                            dst_blk = xt[:, kc, sc * P:(sc + 1) * P]
                            if kc % 2 == 0:
                                nc.vector.tensor_copy(dst_blk, pt[:])
                            else:
                                nc.scalar.copy(dst_blk, pt[:])
                    return xt

                def project_T(xt, w_sb, dst, bias_sb):
                    # dst[p, mc, s] = (X @ W)[s, mc*128+p] (+ bias)
                    # two accumulation groups interleaved so consecutive PE
                    # matmuls target different PSUM banks (pipelines better
                    # than an 8-deep same-bank chain)
                    for sh in range(2):
                        ssl = slice(sh * 512, (sh + 1) * 512)
                        for mc0 in range(0, DC, 2):
                            psA = pj_ps.tile([P, 512], f32, tag="pj", name="psA")
                            psB = pj_ps.tile([P, 512], f32, tag="pj", name="psB")
                            for kc in range(DC):
                                for ps, mc in ((psA, mc0), (psB, mc0 + 1)):
                                    nc.tensor.matmul(
                                        ps[:],
                                        r32(w_sb[:, kc, mc * P:(mc + 1) * P]),
                                        r32(xt[:, kc, ssl]),
                                        start=(kc == 0),
                                        stop=(kc == DC - 1),
                                    )
                            for i, (ps, mc) in enumerate(((psA, mc0), (psB, mc0 + 1))):
                                d = dst[:, mc, ssl]
                                if bias_sb is not None:
                                    nc.vector.tensor_scalar_add(
                                        d, in0=ps[:], scalar1=bias_sb[:, mc:mc + 1]
                                    )
                                elif i == 0:
                                    nc.vector.tensor_copy(d, ps[:])
                                else:
                                    nc.scalar.copy(d, ps[:])

                def project_V(xt, w_sb):
                    # vx[p, sc, h, d] = (Xv @ Wv)[sc*128+p, h*64+d] (+ bias)
                    for sc in range(SC):
                        psA = pj_ps.tile([P, 512], f32, tag="pj", name="psA")
                        psB = pj_ps.tile([P, 512], f32, tag="pj", name="psB")
                        for kc in range(DC):
                            for ps, nh in ((psA, 0), (psB, 1)):
                                nc.tensor.matmul(
                                    ps[:],
                                    r32(xt[:, kc, sc * P:(sc + 1) * P]),
                                    r32(w_sb[:, kc, nh * 512:(nh + 1) * 512]),
                                    start=(kc == 0),
                                    stop=(kc == DC - 1) and not use_bv,
                                )
                        if use_bv:
                            for ps, nh in ((psA, 0), (psB, 1)):
                                nc.tensor.matmul(
                                    ps[:],
                                    ones1[:],
                                    r32(bv_sb[0:1, nh * 512:(nh + 1) * 512]),
                                    start=False,
                                    stop=True,
                                )
                        for i, (ps, nh) in enumerate(((psA, 0), (psB, 1))):
                            dst = vx[:, sc, nh * 8:(nh + 1) * 8, 0:DH]
                            srcp = ps[:].rearrange("p (h d) -> p h d", d=DH)
                            if i == 0:
                                nc.vector.tensor_copy(dst, srcp)
                            else:
                                nc.scalar.copy(dst, srcp)

                with nc.named_scope("proj_k"):
                    xtk = transpose_in(xk)
                    w_sb = load_w(wk)
                    project_T(xtk, w_sb, kt, bk_sb if use_bk else None)
                with nc.named_scope("proj_v"):
                    xtv = transpose_in(xv)
                    w_sb = load_w(wv)
                    project_V(xtv, w_sb)
                with nc.named_scope("proj_q"):
                    xtq = transpose_in(xq)
                    w_sb = load_w(wq)
                    project_T(xtq, w_sb, qt, bq_sb if use_bq else None)

            if debug:
                nc.sync.dma_start(out=d_qt[:], in_=qt[:].bitcast(f32))
                nc.sync.dma_start(out=d_kt[:], in_=kt[:].bitcast(f32))
                nc.sync.dma_start(out=d_vx[:], in_=vx[:].bitcast(f32))

            # ---------------- Phase B: attention ----------------
            ct = bigp.tile([P, DC, S], f32r, tag="big")
            # prefetch Wo during attention on the SWDGE queue (keeps the
            # HWDGE queue free for the latency-critical denom DMAs)
            wo_pref = wp.tile([P, DC, D], f32r, tag="w", name="wo_pref")
            wo_sb_holder = [wo_pref]
            for kc in range(DC):
                nc.gpsimd.dma_start(
                    out=wo_sb_holder[0][:, kc, :],
                    in_=wo[kc * P:(kc + 1) * P, :].bitcast(f32r),
                )
            with tc.tile_pool(name="esp", bufs=4) as esp, \
                 tc.tile_pool(name="rcp", bufs=2) as rcp, \
                 tc.tile_pool(name="tmp", bufs=2) as tmpp, \
                 tc.tile_pool(name="ps_s", bufs=2, space="PSUM") as ps_s, \
                 tc.tile_pool(name="ps_c", bufs=3, space="PSUM") as ps_c, \
                 nc.named_scope("attention"):
                for hp_i in range(HP):
                    hA, hB = 2 * hp_i, 2 * hp_i + 1
                    for qh in range(2):
                        qsl = slice(qh * 512, (qh + 1) * 512)
                        pcA = ps_c.tile([DH + 1, 512], f32, tag="pc")
                        pcB = ps_c.tile([DH + 1, 512], f32, tag="pc")
                        es_tiles = [None] * SC

                        def emit_scores(kt_i):
                            ks = slice(kt_i * P, (kt_i + 1) * P)
                            ps = ps_s.tile([P, 1024], f32, tag="ps")
                            nc.tensor.matmul(
                                ps[:, 0:512],
                                kt[0:64, hp_i, ks],
                                qt[0:64, hp_i, qsl],
                                start=True, stop=True,
                                tile_position=(0, 0),
                            )
                            nc.tensor.matmul(
                                ps[:, 512:1024],
                                kt[64:128, hp_i, ks],
                                qt[64:128, hp_i, qsl],
                                start=True, stop=True,
                                tile_position=(64, 0),
                            )
                            es = esp.tile([P, 1024], bf16, tag="es")
                            nc.scalar.activation(es[:], ps[:], AF.Exp, scale=0.125)
                            es_tiles[kt_i] = es

                        def emit_uctx(kt_i):
                            es = es_tiles[kt_i]
                            nc.tensor.matmul(
                                pcA[:],
                                vx[:, kt_i, hA, :],
                                es[:, 0:512],
                                start=(kt_i == 0), stop=(kt_i == SC - 1),
                            )
                            nc.tensor.matmul(
                                pcB[:],
                                vx[:, kt_i, hB, :],
                                es[:, 512:1024],
                                start=(kt_i == 0), stop=(kt_i == SC - 1),
                            )

                        # software-pipelined: scores run 2 k-tiles ahead of
                        # uctx; uctx is emitted FIRST so the PE has ready work
                        # queued ahead of a scores LDWEIGHTS that may still be
                        # waiting for exp to free its PSUM slot
                        for kt_i in range(SC + 2):
                            if kt_i >= 2:
                                emit_uctx(kt_i - 2)
                            if kt_i < SC:
                                emit_scores(kt_i)

                        if debug and hp_i == 0 and qh == 0:
                            nc.sync.dma_start(out=d_es[:], in_=es_tiles[0][:].bitcast(f32))
                            dpc = tmpp.tile([DH + 1, 512], f32, tag="dpc")
                            nc.vector.tensor_copy(dpc[:], pcA[:])
                            nc.sync.dma_start(out=d_pc[:], in_=dpc[:])

                        # normalize: ctx^T = uctx^T * (1/denom) broadcast
                        for h, pc in ((hA, pcA), (hB, pcB)):
                            # recip on the denom row (partition-aligned with
                            # PSUM row 64), DMA it to partition 0 (HW
                            # partition_broadcast only reads partition 0),
                            # then broadcast to 64 partitions
                            dn = rcp.tile([DH + 1, 512], f32, tag="dn")
                            nc.vector.tensor_copy(dn[DH:DH + 1, :], pc[DH:DH + 1, :])
                            dn0 = rcp.tile([1, 512], f32, tag="dn0")
                            nc.sync.dma_start(out=dn0[:], in_=dn[DH:DH + 1, :])
                            rbc = rcp.tile([DH, 512], f32, tag="rbc")
                            nc.gpsimd.partition_broadcast(rbc[:], dn0[:])
                            # fast recip on SBUF (PSUM reads break its
                            # bitwise seed); in-place on the broadcast tile
                            nc.vector.reciprocal_approx_fast(
                                out=rbc[:], in_=rbc[:]
                            )
                            if h % 2 == 0:
                                nc.vector.tensor_mul(
                                    ct[0:64, hp_i, qsl], pc[0:DH, :], rbc[:]
                                )
                            else:
                                tmp = tmpp.tile([DH, 512], f32r, tag="tmp")
                                nc.vector.tensor_mul(tmp[:], pc[0:DH, :], rbc[:])
                                nc.sync.dma_start(
                                    out=ct[64:128, hp_i, qsl], in_=tmp[:]
                                )

        # ---------------- Phase C: output projection + gelu + residual + LN ----
        with tc.tile_pool(name="yp", bufs=SC) as yp, \
             tc.tile_pool(name="xn2", bufs=3) as xn2, \
             tc.tile_pool(name="stp", bufs=2) as stp, \
             tc.tile_pool(name="mvp", bufs=1) as mvp, \
             tc.tile_pool(name="ps_o", bufs=4, space="PSUM") as ps_o, \
             nc.named_scope("out_proj"):
            if debug:
                nc.sync.dma_start(out=d_ct[:], in_=ct[:].bitcast(f32))
            wo_sb = wo_sb_holder[0]
            mv_all = mvp.tile([P, SC, 2], f32, tag="mv")
            rstd = mvp.tile([P, SC], f32, tag="rstd")
            y_tiles = []

            def emit_chunk(sc):
                ssl = slice(sc * P, (sc + 1) * P)
                xn = xn2.tile([P, D], f32, tag="xn2")
                nc.sync.dma_start(out=xn[:], in_=xq[ssl, :])
                y = yp.tile([P, D], f32, tag="y")
                y_tiles.append(y)
                poA = ps_o.tile([P, 512], f32, tag="po", name="poA")
                poB = ps_o.tile([P, 512], f32, tag="po", name="poB")
                for mc in range(DC):
                    for po, nh in ((poA, 0), (poB, 1)):
                        nc.tensor.matmul(
                            po[:],
                            ct[:, mc, ssl],
                            wo_sb[:, mc, nh * 512:(nh + 1) * 512],
                            start=(mc == 0),
                            stop=(mc == DC - 1) and not use_bo,
                        )
                if use_bo:
                    for po, nh in ((poA, 0), (poB, 1)):
                        nc.tensor.matmul(
                            po[:],
                            ones1[:],
                            bo_sb[0:1, nh * 512:(nh + 1) * 512],
                            start=False, stop=True,
                        )
                for po, nh in ((poA, 0), (poB, 1)):
                    nsl = slice(nh * 512, (nh + 1) * 512)
                    # gelu straight from PSUM, then add residual
                    nc.scalar.activation(y[:, nsl], po[:], AF.Gelu)
                    nc.vector.tensor_add(y[:, nsl], y[:, nsl], xn[:, nsl])
                st = stp.tile([P, 2, 6], f32, tag="st")
                nc.vector.bn_stats(st[:, 0, :], y[:, 0:512])
                nc.vector.bn_stats(st[:, 1, :], y[:, 512:1024])
                nc.vector.bn_aggr(mv_all[:, sc, :], st[:])

            def emit_finalize(batch):
                # rstd for this batch of chunks, then normalize + store
                bsl = slice(batch[0], batch[-1] + 1)
                nc.scalar.activation(
                    rstd[:, bsl], mv_all[:, bsl, 1], AF.Sqrt, bias=eps_sb[:]
                )
                nc.vector.reciprocal(rstd[:, bsl], rstd[:, bsl])
                for sc in batch:
                    y = y_tiles[sc]
                    nc.vector.tensor_scalar(
                        out=y[:],
                        in0=y[:],
                        scalar1=mv_all[:, sc, 0:1],
                        scalar2=rstd[:, sc:sc + 1],
                        op0=Alu.subtract,
                        op1=Alu.mult,
                    )
                    if use_gam:
                        nc.vector.tensor_mul(y[:], y[:], gam_bc[:])
                    if use_bet:
                        nc.vector.tensor_add(y[:], y[:], bet_bc[:])
                    nc.sync.dma_start(out=out[sc * P:(sc + 1) * P, :], in_=y[:])

            for sc in range(5):
                emit_chunk(sc)
            emit_finalize(list(range(5)))
            for sc in range(5, SC):
                emit_chunk(sc)
            emit_finalize(list(range(5, SC)))

    nc.finalize()
    return nc


def _get_nc(flags):
    if flags not in _cache:
        _cache[flags] = _build(flags)
    return _cache[flags]


def kernel(q, k, v, wq, bq, wk, bk, wv, bv, wo, bo, ln_gamma, ln_beta):
    from concourse.bass_utils import run_bass_kernel_spmd

    q = np.ascontiguousarray(q, dtype=np.float32)
    k = np.ascontiguousarray(k, dtype=np.float32)
    v = np.ascontiguousarray(v, dtype=np.float32)

    flags = (
        bool(np.any(bq)), bool(np.any(bk)), bool(np.any(bv)), bool(np.any(bo)),
        not bool(np.all(ln_gamma == 1.0)), bool(np.any(ln_beta)),
    )
    nc = _get_nc(flags)

    shared = {
        "wq": np.ascontiguousarray(wq, np.float32),
        "wk": np.ascontiguousarray(wk, np.float32),
        "wv": np.ascontiguousarray(wv, np.float32),
        "wo": np.ascontiguousarray(wo, np.float32),
        "bq": np.ascontiguousarray(bq, np.float32),
        "bk": np.ascontiguousarray(bk, np.float32),
        "bv": np.ascontiguousarray(bv, np.float32),
        "bo": np.ascontiguousarray(bo, np.float32),
        "gam": np.ascontiguousarray(ln_gamma, np.float32),
        "bet": np.ascontiguousarray(ln_beta, np.float32),
    }
    in_maps = [
        {"xq": q[b], "xk": k[b], "xv": v[b], **shared} for b in range(NCORES)
    ]
    res = run_bass_kernel_spmd(nc, in_maps, core_ids=list(range(NCORES)))
    return np.stack([res.results[b]["out"] for b in range(NCORES)], axis=0)


# revision 24
# speedup vs baseline: 1.0263x; 1.0263x over previous
"""Trainium2 Bass kernel for a full MHA transformer block.

Reference computation (per batch element, data-parallel over 8 NeuronCores):
    qh/kh/vh = (x @ W + b) split into 16 heads of 64
    attn     = softmax(qh @ kh^T / sqrt(64))
    ctx      = attn @ vh
    out      = LayerNorm(gelu(ctx @ Wo + bo) + residual) * gamma + beta

Shapes: B=8, S=1024, D=1024, H=16, DH=64, fp32.

Layout strategy (per core):
  - Inputs are transposed on-chip (PE transpose) to X^T [d, s] so every GEMM
    keeps its contraction dim on SBUF partitions.
  - Q, K are produced transposed (Q^T/K^T: [feature, s]); V natural [s, feature].
  - Scores are computed transposed (scores^T [k, q]) two heads at a time using
    PE row-tiling (each head only uses 64 contraction rows).
  - exp(x/8) on ScalarE straight out of PSUM; no max-subtraction (scores are
    small by construction, fp32 exp is safe).
  - ctx^T [d, q] = V_ext^T-style matmul with an appended ones column producing
    the softmax denominator for free; normalization via a broadcast reciprocal.
  - ctx^T feeds the output projection as the stationary operand, producing the
    output in natural [s, d] layout for gelu/residual/LayerNorm.
"""

import numpy as np

S, D, H, DH = 1024, 1024, 16, 64
EPS = 1e-5
NCORES = 8
P = 128
SC = S // P    # seq chunks (8)
DC = D // P    # feature chunks (8)
HP = H // 2    # head pairs (8)

_cache = {}


def _build(flags, debug=False):
    from contextlib import ExitStack

    import concourse.bass as bass
    import concourse.mybir as mybir
    import concourse.tile as tile
    from concourse import bacc
    from concourse.masks import make_identity

    f32 = mybir.dt.float32
    f32r = mybir.dt.float32r
    bf16 = mybir.dt.bfloat16
    AF = mybir.ActivationFunctionType
    Alu = mybir.AluOpType

    use_bq, use_bk, use_bv, use_bo, use_gam, use_bet = flags

    nc = bacc.Bacc(None, target_bir_lowering=False)

    xq = nc.dram_tensor("xq", [S, D], f32, kind="ExternalInput")
    xk = nc.dram_tensor("xk", [S, D], f32, kind="ExternalInput")
    xv = nc.dram_tensor("xv", [S, D], f32, kind="ExternalInput")
    wq = nc.dram_tensor("wq", [D, D], f32, kind="ExternalInput")
    wk = nc.dram_tensor("wk", [D, D], f32, kind="ExternalInput")
    wv = nc.dram_tensor("wv", [D, D], f32, kind="ExternalInput")
    wo = nc.dram_tensor("wo", [D, D], f32, kind="ExternalInput")
    bq = nc.dram_tensor("bq", [D], f32, kind="ExternalInput")
    bk = nc.dram_tensor("bk", [D], f32, kind="ExternalInput")
    bv = nc.dram_tensor("bv", [D], f32, kind="ExternalInput")
    bo = nc.dram_tensor("bo", [D], f32, kind="ExternalInput")
    gam = nc.dram_tensor("gam", [D], f32, kind="ExternalInput")
    bet = nc.dram_tensor("bet", [D], f32, kind="ExternalInput")
    out = nc.dram_tensor("out", [S, D], f32, kind="ExternalOutput")
    if debug:
        d_qt = nc.dram_tensor("d_qt", [P, DC, S], f32, kind="ExternalOutput")
        d_kt = nc.dram_tensor("d_kt", [P, DC, S], f32, kind="ExternalOutput")
        d_vx = nc.dram_tensor("d_vx", [P, SC, H, DH + 1], f32, kind="ExternalOutput")
        d_es = nc.dram_tensor("d_es", [P, 1024], f32, kind="ExternalOutput")
        d_pc = nc.dram_tensor("d_pc", [DH + 1, 512], f32, kind="ExternalOutput")
        d_ct = nc.dram_tensor("d_ct", [P, DC, S], f32, kind="ExternalOutput")

    def r32(ap):
        return ap.bitcast(f32r)

    with tile.TileContext(nc) as tc, ExitStack() as top:
        consts = top.enter_context(tc.tile_pool(name="consts", bufs=1))
        bigp = top.enter_context(tc.tile_pool(name="bigp", bufs=1))
        wp = top.enter_context(tc.tile_pool(name="wp", bufs=1))

        ident = consts.tile([P, P], f32, tag="ident")
        make_identity(nc, ident[:])

        need_ones = use_bv or use_bo
        if need_ones:
            ones1 = consts.tile([1, P], f32r, tag="ones1")
            nc.vector.memset(ones1[:], 1.0)
        if use_bq:
            bq_sb = consts.tile([P, DC], f32, tag="bq")
            nc.sync.dma_start(out=bq_sb[:], in_=bq[:].rearrange("(c p) -> p c", p=P))
        if use_bk:
            bk_sb = consts.tile([P, DC], f32, tag="bk")
            nc.sync.dma_start(out=bk_sb[:], in_=bk[:].rearrange("(c p) -> p c", p=P))
        if use_bv:
            bv_sb = consts.tile([1, D], f32r, tag="bv")
            nc.sync.dma_start(out=bv_sb[:], in_=bv[:].rearrange("d -> 1 d").bitcast(f32r))
        if use_bo:
            bo_sb = consts.tile([1, D], f32r, tag="bo")
            nc.sync.dma_start(out=bo_sb[:], in_=bo[:].rearrange("d -> 1 d").bitcast(f32r))
        if use_gam:
            gam_bc = consts.tile([P, D], f32, tag="gam")
            nc.sync.dma_start(
                out=gam_bc[:],
                in_=bass.AP(tensor=gam[:].tensor, offset=0, ap=[[0, P], [1, D]]),
            )
        if use_bet:
            bet_bc = consts.tile([P, D], f32, tag="bet")
            nc.sync.dma_start(
                out=bet_bc[:],
                in_=bass.AP(tensor=bet[:].tensor, offset=0, ap=[[0, P], [1, D]]),
            )
        eps_sb = consts.tile([P, 1], f32, tag="eps")
        nc.vector.memset(eps_sb[:], EPS)

        def load_w(wd):
            w_sb = wp.tile([P, DC, D], f32r, tag="w")
            for kc in range(DC):
                nc.sync.dma_start(out=w_sb[:, kc, :], in_=wd[kc * P:(kc + 1) * P, :].bitcast(f32r))
            return w_sb

        with tc.tile_pool(name="qkvp", bufs=1) as qkvp:
            qt = qkvp.tile([P, DC, S], f32r, tag="qt")
            kt = qkvp.tile([P, DC, S], f32r, tag="kt")
            vx = qkvp.tile([P, SC, H, DH + 1], bf16, tag="vx")
            ones16 = consts.tile([P, H], f32, tag="ones16")
            nc.vector.memset(ones16[:], 1.0)
            for sc in range(SC):
                nc.vector.tensor_copy(vx[:, sc, :, DH], ones16[:])

            # ---------------- Phase A: transposes + projections ----------------
            with tc.tile_pool(name="xnp", bufs=3) as xnp, \
                 tc.tile_pool(name="tp_ps", bufs=2, space="PSUM") as tp_ps, \
                 tc.tile_pool(name="pj_ps", bufs=4, space="PSUM") as pj_ps:

                def transpose_in(xd):
                    xt = bigp.tile([P, DC, S], f32r, tag="big")
                    for sc in range(SC):
                        xn = xnp.tile([P, D], f32, tag="xn")
                        nc.sync.dma_start(out=xn[:], in_=xd[sc * P:(sc + 1) * P, :])
                        for kc in range(DC):
                            pt = tp_ps.tile([P, P], f32, tag="tp")
                            nc.tensor.transpose(
                                pt[:], xn[:, kc * P:(kc + 1) * P], ident[:]
                            )
                            dst_blk = xt[:, kc, sc * P:(sc + 1) * P]
                            if kc % 2 == 0:
                                nc.vector.tensor_copy(dst_blk, pt[:])
                            else:
                                nc.scalar.copy(dst_blk, pt[:])
                    return xt

                def project_T(xt, w_sb, dst, bias_sb):
                    # dst[p, mc, s] = (X @ W)[s, mc*128+p] (+ bias)
                    # two accumulation groups interleaved so consecutive PE
                    # matmuls target different PSUM banks (pipelines better
                    # than an 8-deep same-bank chain)
                    for sh in range(2):
                        ssl = slice(sh * 512, (sh + 1) * 512)
                        for mc0 in range(0, DC, 2):
                            psA = pj_ps.tile([P, 512], f32, tag="pj", name="psA")
                            psB = pj_ps.tile([P, 512], f32, tag="pj", name="psB")
                            for kc in range(DC):
                                for ps, mc in ((psA, mc0), (psB, mc0 + 1)):
                                    nc.tensor.matmul(
                                        ps[:],
                                        r32(w_sb[:, kc, mc * P:(mc + 1) * P]),
                                        r32(xt[:, kc, ssl]),
                                        start=(kc == 0),
                                        stop=(kc == DC - 1),
                                    )
                            for i, (ps, mc) in enumerate(((psA, mc0), (psB, mc0 + 1))):
                                d = dst[:, mc, ssl]
                                if bias_sb is not None:
                                    nc.vector.tensor_scalar_add(
                                        d, in0=ps[:], scalar1=bias_sb[:, mc:mc + 1]
                                    )
                                elif i == 0:
                                    nc.vector.tensor_copy(d, ps[:])
                                else:
                                    nc.scalar.copy(d, ps[:])

                def project_V(xt, w_sb):
                    # vx[p, sc, h, d] = (Xv @ Wv)[sc*128+p, h*64+d] (+ bias)
                    for sc in range(SC):
                        psA = pj_ps.tile([P, 512], f32, tag="pj", name="psA")
                        psB = pj_ps.tile([P, 512], f32, tag="pj", name="psB")
                        for kc in range(DC):
                            for ps, nh in ((psA, 0), (psB, 1)):
                                nc.tensor.matmul(
                                    ps[:],
                                    r32(xt[:, kc, sc * P:(sc + 1) * P]),
                                    r32(w_sb[:, kc, nh * 512:(nh + 1) * 512]),
                                    start=(kc == 0),
                                    stop=(kc == DC - 1) and not use_bv,
                                )
                        if use_bv:
                            for ps, nh in ((psA, 0), (psB, 1)):
                                nc.tensor.matmul(
                                    ps[:],
                                    ones1[:],
                                    r32(bv_sb[0:1, nh * 512:(nh + 1) * 512]),
                                    start=False,
                                    stop=True,
                                )
                        for i, (ps, nh) in enumerate(((psA, 0), (psB, 1))):
                            dst = vx[:, sc, nh * 8:(nh + 1) * 8, 0:DH]
                            srcp = ps[:].rearrange("p (h d) -> p h d", d=DH)
                            if i == 0:
                                nc.vector.tensor_copy(dst, srcp)
                            else:
                                nc.scalar.copy(dst, srcp)

                with nc.named_scope("proj_k"):
                    xtk = transpose_in(xk)
                    w_sb = load_w(wk)
                    project_T(xtk, w_sb, kt, bk_sb if use_bk else None)
                with nc.named_scope("proj_v"):
                    xtv = transpose_in(xv)
                    w_sb = load_w(wv)
                    project_V(xtv, w_sb)
                with nc.named_scope("proj_q"):
                    xtq = transpose_in(xq)
                    w_sb = load_w(wq)
                    project_T(xtq, w_sb, qt, bq_sb if use_bq else None)

            if debug:
                nc.sync.dma_start(out=d_qt[:], in_=qt[:].bitcast(f32))
                nc.sync.dma_start(out=d_kt[:], in_=kt[:].bitcast(f32))
                nc.sync.dma_start(out=d_vx[:], in_=vx[:].bitcast(f32))

            # ---------------- Phase B: attention ----------------
            ct = bigp.tile([P, DC, S], f32r, tag="big")
            # prefetch Wo during attention on the SWDGE queue (keeps the
            # HWDGE queue free for the latency-critical denom DMAs)
            wo_pref = wp.tile([P, DC, D], f32r, tag="w", name="wo_pref")
            wo_sb_holder = [wo_pref]
            for kc in range(DC):
                nc.gpsimd.dma_start(
                    out=wo_sb_holder[0][:, kc, :],
                    in_=wo[kc * P:(kc + 1) * P, :].bitcast(f32r),
                )
            with tc.tile_pool(name="esp", bufs=4) as esp, \
                 tc.tile_pool(name="rcp", bufs=2) as rcp, \
                 tc.tile_pool(name="tmp", bufs=2) as tmpp, \
                 tc.tile_pool(name="ps_s", bufs=2, space="PSUM") as ps_s, \
                 tc.tile_pool(name="ps_c", bufs=3, space="PSUM") as ps_c, \
                 nc.named_scope("attention"):
                for hp_i in range(HP):
                    hA, hB = 2 * hp_i, 2 * hp_i + 1
                    for qh in range(2):
                        qsl = slice(qh * 512, (qh + 1) * 512)
                        pcA = ps_c.tile([DH + 1, 512], f32, tag="pc")
                        pcB = ps_c.tile([DH + 1, 512], f32, tag="pc")
                        es_tiles = [None] * SC

                        def emit_scores(kt_i):
                            ks = slice(kt_i * P, (kt_i + 1) * P)
                            ps = ps_s.tile([P, 1024], f32, tag="ps")
                            nc.tensor.matmul(
                                ps[:, 0:512],
                                kt[0:64, hp_i, ks],
                                qt[0:64, hp_i, qsl],
                                start=True, stop=True,
                                tile_position=(0, 0),
                            )
                            nc.tensor.matmul(
                                ps[:, 512:1024],
                                kt[64:128, hp_i, ks],
                                qt[64:128, hp_i, qsl],
                                start=True, stop=True,
                                tile_position=(64, 0),
                            )
                            es = esp.tile([P, 1024], bf16, tag="es")
                            nc.scalar.activation(es[:], ps[:], AF.Exp, scale=0.125)
                            es_tiles[kt_i] = es

                        def emit_uctx(kt_i):
                            es = es_tiles[kt_i]
                            nc.tensor.matmul(
                                pcA[:],
                                vx[:, kt_i, hA, :],
                                es[:, 0:512],
                                start=(kt_i == 0), stop=(kt_i == SC - 1),
                            )
                            nc.tensor.matmul(
                                pcB[:],
                                vx[:, kt_i, hB, :],
                                es[:, 512:1024],
                                start=(kt_i == 0), stop=(kt_i == SC - 1),
                            )

                        # software-pipelined: scores run 2 k-tiles ahead of uctx
                        for kt_i in range(SC + 2):
                            if kt_i < SC:
                                emit_scores(kt_i)
                            if kt_i >= 2:
                                emit_uctx(kt_i - 2)

                        if debug and hp_i == 0 and qh == 0:
                            nc.sync.dma_start(out=d_es[:], in_=es_tiles[0][:].bitcast(f32))
                            dpc = tmpp.tile([DH + 1, 512], f32, tag="dpc")
                            nc.vector.tensor_copy(dpc[:], pcA[:])
                            nc.sync.dma_start(out=d_pc[:], in_=dpc[:])

                        # normalize: ctx^T = uctx^T * (1/denom) broadcast
                        for h, pc in ((hA, pcA), (hB, pcB)):
                            # recip on the denom row (partition-aligned with
                            # PSUM row 64), DMA it to partition 0 (HW
                            # partition_broadcast only reads partition 0),
                            # then broadcast to 64 partitions
                            dn = rcp.tile([DH + 1, 512], f32, tag="dn")
                            nc.vector.tensor_copy(dn[DH:DH + 1, :], pc[DH:DH + 1, :])
                            dn0 = rcp.tile([1, 512], f32, tag="dn0")
                            nc.sync.dma_start(out=dn0[:], in_=dn[DH:DH + 1, :])
                            rbc = rcp.tile([DH, 512], f32, tag="rbc")
                            nc.gpsimd.partition_broadcast(rbc[:], dn0[:])
                            # fast recip on SBUF (PSUM reads break its
                            # bitwise seed); in-place on the broadcast tile
                            nc.vector.reciprocal_approx_fast(
                                out=rbc[:], in_=rbc[:]
                            )
                            if h % 2 == 0:
                                nc.vector.tensor_mul(
                                    ct[0:64, hp_i, qsl], pc[0:DH, :], rbc[:]
                                )
                            else:
                                tmp = tmpp.tile([DH, 512], f32r, tag="tmp")
                                nc.vector.tensor_mul(tmp[:], pc[0:DH, :], rbc[:])
                                nc.sync.dma_start(
                                    out=ct[64:128, hp_i, qsl], in_=tmp[:]
                                )

        # ---------------- Phase C: output projection + gelu + residual + LN ----
        with tc.tile_pool(name="yp", bufs=SC) as yp, \
             tc.tile_pool(name="xn2", bufs=3) as xn2, \
             tc.tile_pool(name="stp", bufs=2) as stp, \
             tc.tile_pool(name="mvp", bufs=1) as mvp, \
             tc.tile_pool(name="ps_o", bufs=4, space="PSUM") as ps_o, \
             nc.named_scope("out_proj"):
            if debug:
                nc.sync.dma_start(out=d_ct[:], in_=ct[:].bitcast(f32))
            wo_sb = wo_sb_holder[0]
            mv_all = mvp.tile([P, SC, 2], f32, tag="mv")
            rstd = mvp.tile([P, SC], f32, tag="rstd")
            y_tiles = []

            def emit_chunk(sc):
                ssl = slice(sc * P, (sc + 1) * P)
                xn = xn2.tile([P, D], f32, tag="xn2")
                nc.sync.dma_start(out=xn[:], in_=xq[ssl, :])
                y = yp.tile([P, D], f32, tag="y")
                y_tiles.append(y)
                poA = ps_o.tile([P, 512], f32, tag="po", name="poA")
                poB = ps_o.tile([P, 512], f32, tag="po", name="poB")
                for mc in range(DC):
                    for po, nh in ((poA, 0), (poB, 1)):
                        nc.tensor.matmul(
                            po[:],
                            ct[:, mc, ssl],
                            wo_sb[:, mc, nh * 512:(nh + 1) * 512],
                            start=(mc == 0),
                            stop=(mc == DC - 1) and not use_bo,
                        )
                if use_bo:
                    for po, nh in ((poA, 0), (poB, 1)):
                        nc.tensor.matmul(
                            po[:],
                            ones1[:],
                            bo_sb[0:1, nh * 512:(nh + 1) * 512],
                            start=False, stop=True,
                        )
                for po, nh in ((poA, 0), (poB, 1)):
                    nsl = slice(nh * 512, (nh + 1) * 512)
                    # gelu straight from PSUM, then add residual
                    nc.scalar.activation(y[:, nsl], po[:], AF.Gelu)
                    nc.vector.tensor_add(y[:, nsl], y[:, nsl], xn[:, nsl])
                st = stp.tile([P, 2, 6], f32, tag="st")
                nc.vector.bn_stats(st[:, 0, :], y[:, 0:512])
                nc.vector.bn_stats(st[:, 1, :], y[:, 512:1024])
                nc.vector.bn_aggr(mv_all[:, sc, :], st[:])

            def emit_finalize(batch):
                # rstd for this batch of chunks, then normalize + store
                bsl = slice(batch[0], batch[-1] + 1)
                nc.scalar.activation(
                    rstd[:, bsl], mv_all[:, bsl, 1], AF.Sqrt, bias=eps_sb[:]
                )
                nc.vector.reciprocal(rstd[:, bsl], rstd[:, bsl])
                for sc in batch:
                    y = y_tiles[sc]
                    nc.vector.tensor_scalar(
                        out=y[:],
                        in0=y[:],
                        scalar1=mv_all[:, sc, 0:1],
                        scalar2=rstd[:, sc:sc + 1],
                        op0=Alu.subtract,
                        op1=Alu.mult,
                    )
                    if use_gam:
                        nc.vector.tensor_mul(y[:], y[:], gam_bc[:])
                    if use_bet:
                        nc.vector.tensor_add(y[:], y[:], bet_bc[:])
                    nc.sync.dma_start(out=out[sc * P:(sc + 1) * P, :], in_=y[:])

            for sc in range(5):
                emit_chunk(sc)
            emit_finalize(list(range(5)))
            for sc in range(5, SC):
                emit_chunk(sc)
            emit_finalize(list(range(5, SC)))

    nc.finalize()
    return nc


def _get_nc(flags):
    if flags not in _cache:
        _cache[flags] = _build(flags)
    return _cache[flags]


def kernel(q, k, v, wq, bq, wk, bk, wv, bv, wo, bo, ln_gamma, ln_beta):
    from concourse.bass_utils import run_bass_kernel_spmd

    q = np.ascontiguousarray(q, dtype=np.float32)
    k = np.ascontiguousarray(k, dtype=np.float32)
    v = np.ascontiguousarray(v, dtype=np.float32)

    flags = (
        bool(np.any(bq)), bool(np.any(bk)), bool(np.any(bv)), bool(np.any(bo)),
        not bool(np.all(ln_gamma == 1.0)), bool(np.any(ln_beta)),
    )
    nc = _get_nc(flags)

    shared = {
        "wq": np.ascontiguousarray(wq, np.float32),
        "wk": np.ascontiguousarray(wk, np.float32),
        "wv": np.ascontiguousarray(wv, np.float32),
        "wo": np.ascontiguousarray(wo, np.float32),
        "bq": np.ascontiguousarray(bq, np.float32),
        "bk": np.ascontiguousarray(bk, np.float32),
        "bv": np.ascontiguousarray(bv, np.float32),
        "bo": np.ascontiguousarray(bo, np.float32),
        "gam": np.ascontiguousarray(ln_gamma, np.float32),
        "bet": np.ascontiguousarray(ln_beta, np.float32),
    }
    in_maps = [
        {"xq": q[b], "xk": k[b], "xv": v[b], **shared} for b in range(NCORES)
    ]
    res = run_bass_kernel_spmd(nc, in_maps, core_ids=list(range(NCORES)))
    return np.stack([res.results[b]["out"] for b in range(NCORES)], axis=0)


# revision 25
# speedup vs baseline: 1.1077x; 1.0793x over previous
"""Trainium2 Bass kernel for a full MHA transformer block.

Reference computation (per batch element, data-parallel over 8 NeuronCores):
    qh/kh/vh = (x @ W + b) split into 16 heads of 64
    attn     = softmax(qh @ kh^T / sqrt(64))
    ctx      = attn @ vh
    out      = LayerNorm(gelu(ctx @ Wo + bo) + residual) * gamma + beta

Shapes: B=8, S=1024, D=1024, H=16, DH=64, fp32.

Layout strategy (per core):
  - Inputs are transposed on-chip (PE transpose) to X^T [d, s] so every GEMM
    keeps its contraction dim on SBUF partitions.
  - Q, K are produced transposed (Q^T/K^T: [feature, s]); V natural [s, feature].
  - Scores are computed transposed (scores^T [k, q]) two heads at a time using
    PE row-tiling (each head only uses 64 contraction rows).
  - exp(x/8) on ScalarE straight out of PSUM; no max-subtraction (scores are
    small by construction, fp32 exp is safe).
  - ctx^T [d, q] = V_ext^T-style matmul with an appended ones column producing
    the softmax denominator for free; normalization via a broadcast reciprocal.
  - ctx^T feeds the output projection as the stationary operand, producing the
    output in natural [s, d] layout for gelu/residual/LayerNorm.
"""

import numpy as np

S, D, H, DH = 1024, 1024, 16, 64
EPS = 1e-5
NCORES = 8
P = 128
SC = S // P    # seq chunks (8)
DC = D // P    # feature chunks (8)
HP = H // 2    # head pairs (8)

_cache = {}


def _build(flags, debug=False):
    from contextlib import ExitStack

    import concourse.bass as bass
    import concourse.mybir as mybir
    import concourse.tile as tile
    from concourse import bacc
    from concourse.masks import make_identity

    f32 = mybir.dt.float32
    f32r = mybir.dt.float32r
    bf16 = mybir.dt.bfloat16
    AF = mybir.ActivationFunctionType
    Alu = mybir.AluOpType

    use_bq, use_bk, use_bv, use_bo, use_gam, use_bet = flags

    nc = bacc.Bacc(None, target_bir_lowering=False)

    xq = nc.dram_tensor("xq", [S, D], f32, kind="ExternalInput")
    xk = nc.dram_tensor("xk", [S, D], f32, kind="ExternalInput")
    xv = nc.dram_tensor("xv", [S, D], f32, kind="ExternalInput")
    wq = nc.dram_tensor("wq", [D, D], f32, kind="ExternalInput")
    wk = nc.dram_tensor("wk", [D, D], f32, kind="ExternalInput")
    wv = nc.dram_tensor("wv", [D, D], f32, kind="ExternalInput")
    wo = nc.dram_tensor("wo", [D, D], f32, kind="ExternalInput")
    bq = nc.dram_tensor("bq", [D], f32, kind="ExternalInput")
    bk = nc.dram_tensor("bk", [D], f32, kind="ExternalInput")
    bv = nc.dram_tensor("bv", [D], f32, kind="ExternalInput")
    bo = nc.dram_tensor("bo", [D], f32, kind="ExternalInput")
    gam = nc.dram_tensor("gam", [D], f32, kind="ExternalInput")
    bet = nc.dram_tensor("bet", [D], f32, kind="ExternalInput")
    out = nc.dram_tensor("out", [S, D], f32, kind="ExternalOutput")
    if debug:
        d_qt = nc.dram_tensor("d_qt", [P, DC, S], f32, kind="ExternalOutput")
        d_kt = nc.dram_tensor("d_kt", [P, DC, S], f32, kind="ExternalOutput")
        d_vx = nc.dram_tensor("d_vx", [P, SC, H, DH + 1], f32, kind="ExternalOutput")
        d_es = nc.dram_tensor("d_es", [P, 1024], f32, kind="ExternalOutput")
        d_pc = nc.dram_tensor("d_pc", [DH + 1, 512], f32, kind="ExternalOutput")
        d_ct = nc.dram_tensor("d_ct", [P, DC, S], f32, kind="ExternalOutput")

    def r32(ap):
        return ap.bitcast(f32r)

    with tile.TileContext(nc) as tc, ExitStack() as top:
        consts = top.enter_context(tc.tile_pool(name="consts", bufs=1))
        bigp = top.enter_context(tc.tile_pool(name="bigp", bufs=1))
        wp = top.enter_context(tc.tile_pool(name="wp", bufs=1))

        ident = consts.tile([P, P], f32, tag="ident")
        make_identity(nc, ident[:])

        need_ones = use_bv or use_bo
        if need_ones:
            ones1 = consts.tile([1, P], f32r, tag="ones1")
            nc.vector.memset(ones1[:], 1.0)
        if use_bq:
            bq_sb = consts.tile([P, DC], f32, tag="bq")
            nc.sync.dma_start(out=bq_sb[:], in_=bq[:].rearrange("(c p) -> p c", p=P))
        if use_bk:
            bk_sb = consts.tile([P, DC], f32, tag="bk")
            nc.sync.dma_start(out=bk_sb[:], in_=bk[:].rearrange("(c p) -> p c", p=P))
        if use_bv:
            bv_sb = consts.tile([1, D], f32r, tag="bv")
            nc.sync.dma_start(out=bv_sb[:], in_=bv[:].rearrange("d -> 1 d").bitcast(f32r))
        if use_bo:
            bo_sb = consts.tile([1, D], f32r, tag="bo")
            nc.sync.dma_start(out=bo_sb[:], in_=bo[:].rearrange("d -> 1 d").bitcast(f32r))
        if use_gam:
            gam_bc = consts.tile([P, D], f32, tag="gam")
            nc.sync.dma_start(
                out=gam_bc[:],
                in_=bass.AP(tensor=gam[:].tensor, offset=0, ap=[[0, P], [1, D]]),
            )
        if use_bet:
            bet_bc = consts.tile([P, D], f32, tag="bet")
            nc.sync.dma_start(
                out=bet_bc[:],
                in_=bass.AP(tensor=bet[:].tensor, offset=0, ap=[[0, P], [1, D]]),
            )
        eps_sb = consts.tile([P, 1], f32, tag="eps")
        nc.vector.memset(eps_sb[:], EPS)

        def load_w(wd):
            w_sb = wp.tile([P, DC, D], f32r, tag="w")
            for kc in range(DC):
                nc.sync.dma_start(out=w_sb[:, kc, :], in_=wd[kc * P:(kc + 1) * P, :].bitcast(f32r))
            return w_sb

        with tc.tile_pool(name="qkvp", bufs=1) as qkvp:
            qt = qkvp.tile([P, DC, S], f32r, tag="qt")
            kt = qkvp.tile([P, DC, S], f32r, tag="kt")
            vx = qkvp.tile([P, SC, H, DH + 1], bf16, tag="vx")
            ones16 = consts.tile([P, H], f32, tag="ones16")
            nc.vector.memset(ones16[:], 1.0)
            for sc in range(SC):
                nc.vector.tensor_copy(vx[:, sc, :, DH], ones16[:])

            # ---------------- Phase A: transposes + projections ----------------
            with tc.tile_pool(name="xnp", bufs=4) as xnp, \
                 tc.tile_pool(name="tp_ps", bufs=4, space="PSUM") as tp_ps, \
                 tc.tile_pool(name="pj_ps", bufs=4, space="PSUM") as pj_ps:

                def transpose_in(xd):
                    xt = bigp.tile([P, DC, S], f32r, tag="big")
                    for sc in range(SC):
                        xn = xnp.tile([P, D], f32, tag="xn")
                        nc.sync.dma_start(out=xn[:], in_=xd[sc * P:(sc + 1) * P, :])
                        for kc in range(DC):
                            pt = tp_ps.tile([P, P], f32, tag="tp")
                            nc.tensor.transpose(
                                pt[:], xn[:, kc * P:(kc + 1) * P], ident[:]
                            )
                            dst_blk = xt[:, kc, sc * P:(sc + 1) * P]
                            if kc % 2 == 0:
                                nc.vector.tensor_copy(dst_blk, pt[:])
                            else:
                                nc.scalar.copy(dst_blk, pt[:])
                    return xt

                def project_T(xt, w_sb, dst, bias_sb):
                    # dst[p, mc, s] = (X @ W)[s, mc*128+p] (+ bias)
                    # two accumulation groups interleaved so consecutive PE
                    # matmuls target different PSUM banks (pipelines better
                    # than an 8-deep same-bank chain)
                    for sh in range(2):
                        ssl = slice(sh * 512, (sh + 1) * 512)
                        for mc0 in range(0, DC, 2):
                            psA = pj_ps.tile([P, 512], f32, tag="pj", name="psA")
                            psB = pj_ps.tile([P, 512], f32, tag="pj", name="psB")
                            for kc in range(DC):
                                for ps, mc in ((psA, mc0), (psB, mc0 + 1)):
                                    nc.tensor.matmul(
                                        ps[:],
                                        r32(w_sb[:, kc, mc * P:(mc + 1) * P]),
                                        r32(xt[:, kc, ssl]),
                                        start=(kc == 0),
                                        stop=(kc == DC - 1),
                                    )
                            for i, (ps, mc) in enumerate(((psA, mc0), (psB, mc0 + 1))):
                                d = dst[:, mc, ssl]
                                if bias_sb is not None:
                                    nc.vector.tensor_scalar_add(
                                        d, in0=ps[:], scalar1=bias_sb[:, mc:mc + 1]
                                    )
                                elif i == 0:
                                    nc.vector.tensor_copy(d, ps[:])
                                else:
                                    nc.scalar.copy(d, ps[:])

                def project_V(xt, w_sb):
                    # vx[p, sc, h, d] = (Xv @ Wv)[sc*128+p, h*64+d] (+ bias)
                    for sc in range(SC):
                        psA = pj_ps.tile([P, 512], f32, tag="pj", name="psA")
                        psB = pj_ps.tile([P, 512], f32, tag="pj", name="psB")
                        for kc in range(DC):
                            for ps, nh in ((psA, 0), (psB, 1)):
                                nc.tensor.matmul(
                                    ps[:],
                                    r32(xt[:, kc, sc * P:(sc + 1) * P]),
                                    r32(w_sb[:, kc, nh * 512:(nh + 1) * 512]),
                                    start=(kc == 0),
                                    stop=(kc == DC - 1) and not use_bv,
                                )
                        if use_bv:
                            for ps, nh in ((psA, 0), (psB, 1)):
                                nc.tensor.matmul(
                                    ps[:],
                                    ones1[:],
                                    r32(bv_sb[0:1, nh * 512:(nh + 1) * 512]),
                                    start=False,
                                    stop=True,
                                )
                        for i, (ps, nh) in enumerate(((psA, 0), (psB, 1))):
                            dst = vx[:, sc, nh * 8:(nh + 1) * 8, 0:DH]
                            srcp = ps[:].rearrange("p (h d) -> p h d", d=DH)
                            if i == 0:
                                nc.vector.tensor_copy(dst, srcp)
                            else:
                                nc.scalar.copy(dst, srcp)

                with nc.named_scope("proj_k"):
                    xtk = transpose_in(xk)
                    w_sb = load_w(wk)
                    project_T(xtk, w_sb, kt, bk_sb if use_bk else None)
                with nc.named_scope("proj_v"):
                    xtv = transpose_in(xv)
                    w_sb = load_w(wv)
                    project_V(xtv, w_sb)
                with nc.named_scope("proj_q"):
                    xtq = transpose_in(xq)
                    w_sb = load_w(wq)
                    project_T(xtq, w_sb, qt, bq_sb if use_bq else None)

            if debug:
                nc.sync.dma_start(out=d_qt[:], in_=qt[:].bitcast(f32))
                nc.sync.dma_start(out=d_kt[:], in_=kt[:].bitcast(f32))
                nc.sync.dma_start(out=d_vx[:], in_=vx[:].bitcast(f32))

            # ---------------- Phase B: attention ----------------
            ct = bigp.tile([P, DC, S], f32r, tag="big")
            # prefetch Wo during attention on the SWDGE queue (keeps the
            # HWDGE queue free for the latency-critical denom DMAs)
            wo_pref = wp.tile([P, DC, D], f32r, tag="w", name="wo_pref")
            wo_sb_holder = [wo_pref]
            for kc in range(DC):
                nc.gpsimd.dma_start(
                    out=wo_sb_holder[0][:, kc, :],
                    in_=wo[kc * P:(kc + 1) * P, :].bitcast(f32r),
                )
            with tc.tile_pool(name="esp", bufs=6) as esp, \
                 tc.tile_pool(name="rcp", bufs=2) as rcp, \
                 tc.tile_pool(name="tmp", bufs=2) as tmpp, \
                 tc.tile_pool(name="ps_s", bufs=2, space="PSUM") as ps_s, \
                 tc.tile_pool(name="ps_c", bufs=3, space="PSUM") as ps_c, \
                 nc.named_scope("attention"):
                for hp_i in range(HP):
                    hA, hB = 2 * hp_i, 2 * hp_i + 1
                    for qh in range(2):
                        qsl = slice(qh * 512, (qh + 1) * 512)
                        pcA = ps_c.tile([DH + 1, 512], f32, tag="pc")
                        pcB = ps_c.tile([DH + 1, 512], f32, tag="pc")
                        es_tiles = [None] * SC

                        def emit_scores(kt_i):
                            ks = slice(kt_i * P, (kt_i + 1) * P)
                            ps = ps_s.tile([P, 1024], f32, tag="ps")
                            nc.tensor.matmul(
                                ps[:, 0:512],
                                kt[0:64, hp_i, ks],
                                qt[0:64, hp_i, qsl],
                                start=True, stop=True,
                                tile_position=(0, 0),
                            )
                            nc.tensor.matmul(
                                ps[:, 512:1024],
                                kt[64:128, hp_i, ks],
                                qt[64:128, hp_i, qsl],
                                start=True, stop=True,
                                tile_position=(64, 0),
                            )
                            es = esp.tile([P, 1024], bf16, tag="es")
                            nc.scalar.activation(es[:], ps[:], AF.Exp, scale=0.125)
                            es_tiles[kt_i] = es

                        def emit_uctx(kt_i):
                            es = es_tiles[kt_i]
                            nc.tensor.matmul(
                                pcA[:],
                                vx[:, kt_i, hA, :],
                                es[:, 0:512],
                                start=(kt_i == 0), stop=(kt_i == SC - 1),
                            )
                            nc.tensor.matmul(
                                pcB[:],
                                vx[:, kt_i, hB, :],
                                es[:, 512:1024],
                                start=(kt_i == 0), stop=(kt_i == SC - 1),
                            )

                        # software-pipelined: scores run 2 k-tiles ahead of uctx
                        for kt_i in range(SC + 2):
                            if kt_i < SC:
                                emit_scores(kt_i)
                            if kt_i >= 2:
                                emit_uctx(kt_i - 2)

                        if debug and hp_i == 0 and qh == 0:
                            nc.sync.dma_start(out=d_es[:], in_=es_tiles[0][:].bitcast(f32))
                            dpc = tmpp.tile([DH + 1, 512], f32, tag="dpc")
                            nc.vector.tensor_copy(dpc[:], pcA[:])
                            nc.sync.dma_start(out=d_pc[:], in_=dpc[:])

                        # normalize: ctx^T = uctx^T * (1/denom) broadcast
                        for h, pc in ((hA, pcA), (hB, pcB)):
                            # recip on the denom row (partition-aligned with
                            # PSUM row 64), DMA it to partition 0 (HW
                            # partition_broadcast only reads partition 0),
                            # then broadcast to 64 partitions
                            dn = rcp.tile([DH + 1, 512], f32, tag="dn")
                            nc.vector.tensor_copy(dn[DH:DH + 1, :], pc[DH:DH + 1, :])
                            dn0 = rcp.tile([1, 512], f32, tag="dn0")
                            nc.sync.dma_start(out=dn0[:], in_=dn[DH:DH + 1, :])
                            rbc = rcp.tile([DH, 512], f32, tag="rbc")
                            nc.gpsimd.partition_broadcast(rbc[:], dn0[:])
                            # fast recip on SBUF (PSUM reads break its
                            # bitwise seed); in-place on the broadcast tile
                            nc.vector.reciprocal_approx_fast(
                                out=rbc[:], in_=rbc[:]
                            )
                            if h % 2 == 0:
                                nc.vector.tensor_mul(
                                    ct[0:64, hp_i, qsl], pc[0:DH, :], rbc[:]
                                )
                            else:
                                tmp = tmpp.tile([DH, 512], f32r, tag="tmp")
                                nc.vector.tensor_mul(tmp[:], pc[0:DH, :], rbc[:])
                                nc.sync.dma_start(
                                    out=ct[64:128, hp_i, qsl], in_=tmp[:]
                                )

        # ---------------- Phase C: output projection + gelu + residual + LN ----
        with tc.tile_pool(name="yp", bufs=SC) as yp, \
             tc.tile_pool(name="xn2", bufs=3) as xn2, \
             tc.tile_pool(name="stp", bufs=2) as stp, \
             tc.tile_pool(name="mvp", bufs=1) as mvp, \
             tc.tile_pool(name="ps_o", bufs=4, space="PSUM") as ps_o, \
             nc.named_scope("out_proj"):
            if debug:
                nc.sync.dma_start(out=d_ct[:], in_=ct[:].bitcast(f32))
            wo_sb = wo_sb_holder[0]
            mv_all = mvp.tile([P, SC, 2], f32, tag="mv")
            rstd = mvp.tile([P, SC], f32, tag="rstd")
            y_tiles = []

            def emit_chunk(sc):
                ssl = slice(sc * P, (sc + 1) * P)
                xn = xn2.tile([P, D], f32, tag="xn2")
                nc.sync.dma_start(out=xn[:], in_=xq[ssl, :])
                y = yp.tile([P, D], f32, tag="y")
                y_tiles.append(y)
                poA = ps_o.tile([P, 512], f32, tag="po", name="poA")
                poB = ps_o.tile([P, 512], f32, tag="po", name="poB")
                for mc in range(DC):
                    for po, nh in ((poA, 0), (poB, 1)):
                        nc.tensor.matmul(
                            po[:],
                            ct[:, mc, ssl],
                            wo_sb[:, mc, nh * 512:(nh + 1) * 512],
                            start=(mc == 0),
                            stop=(mc == DC - 1) and not use_bo,
                        )
                if use_bo:
                    for po, nh in ((poA, 0), (poB, 1)):
                        nc.tensor.matmul(
                            po[:],
                            ones1[:],
                            bo_sb[0:1, nh * 512:(nh + 1) * 512],
                            start=False, stop=True,
                        )
                for po, nh in ((poA, 0), (poB, 1)):
                    nsl = slice(nh * 512, (nh + 1) * 512)
                    # gelu straight from PSUM, then add residual
                    nc.scalar.activation(y[:, nsl], po[:], AF.Gelu)
                    nc.vector.tensor_add(y[:, nsl], y[:, nsl], xn[:, nsl])
                st = stp.tile([P, 2, 6], f32, tag="st")
                nc.vector.bn_stats(st[:, 0, :], y[:, 0:512])
                nc.vector.bn_stats(st[:, 1, :], y[:, 512:1024])
                nc.vector.bn_aggr(mv_all[:, sc, :], st[:])

            def emit_finalize(batch):
                # rstd for this batch of chunks, then normalize + store
                bsl = slice(batch[0], batch[-1] + 1)
                nc.scalar.activation(
                    rstd[:, bsl], mv_all[:, bsl, 1], AF.Sqrt, bias=eps_sb[:]
                )
                nc.vector.reciprocal(rstd[:, bsl], rstd[:, bsl])
                for sc in batch:
                    y = y_tiles[sc]
                    nc.vector.tensor_scalar(
                        out=y[:],
                        in0=y[:],
                        scalar1=mv_all[:, sc, 0:1],
                        scalar2=rstd[:, sc:sc + 1],
                        op0=Alu.subtract,
                        op1=Alu.mult,
                    )
                    if use_gam:
                        nc.vector.tensor_mul(y[:], y[:], gam_bc[:])
                    if use_bet:
                        nc.vector.tensor_add(y[:], y[:], bet_bc[:])
                    nc.sync.dma_start(out=out[sc * P:(sc + 1) * P, :], in_=y[:])

            for sc in range(5):
                emit_chunk(sc)
            emit_finalize(list(range(5)))
            for sc in range(5, SC):
                emit_chunk(sc)
            emit_finalize(list(range(5, SC)))

    nc.finalize()
    return nc


def _get_nc(flags):
    if flags not in _cache:
        _cache[flags] = _build(flags)
    return _cache[flags]


def kernel(q, k, v, wq, bq, wk, bk, wv, bv, wo, bo, ln_gamma, ln_beta):
    from concourse.bass_utils import run_bass_kernel_spmd

    q = np.ascontiguousarray(q, dtype=np.float32)
    k = np.ascontiguousarray(k, dtype=np.float32)
    v = np.ascontiguousarray(v, dtype=np.float32)

    flags = (
        bool(np.any(bq)), bool(np.any(bk)), bool(np.any(bv)), bool(np.any(bo)),
        not bool(np.all(ln_gamma == 1.0)), bool(np.any(ln_beta)),
    )
    nc = _get_nc(flags)

    shared = {
        "wq": np.ascontiguousarray(wq, np.float32),
        "wk": np.ascontiguousarray(wk, np.float32),
        "wv": np.ascontiguousarray(wv, np.float32),
        "wo": np.ascontiguousarray(wo, np.float32),
        "bq": np.ascontiguousarray(bq, np.float32),
        "bk": np.ascontiguousarray(bk, np.float32),
        "bv": np.ascontiguousarray(bv, np.float32),
        "bo": np.ascontiguousarray(bo, np.float32),
        "gam": np.ascontiguousarray(ln_gamma, np.float32),
        "bet": np.ascontiguousarray(ln_beta, np.float32),
    }
    in_maps = [
        {"xq": q[b], "xk": k[b], "xv": v[b], **shared} for b in range(NCORES)
    ]
    res = run_bass_kernel_spmd(nc, in_maps, core_ids=list(range(NCORES)))
    return np.stack([res.results[b]["out"] for b in range(NCORES)], axis=0)


# revision 26
# speedup vs baseline: 1.1414x; 1.0304x over previous
"""Trainium2 Bass kernel for a full MHA transformer block.

Reference computation (per batch element, data-parallel over 8 NeuronCores):
    qh/kh/vh = (x @ W + b) split into 16 heads of 64
    attn     = softmax(qh @ kh^T / sqrt(64))
    ctx      = attn @ vh
    out      = LayerNorm(gelu(ctx @ Wo + bo) + residual) * gamma + beta

Shapes: B=8, S=1024, D=1024, H=16, DH=64, fp32.

Layout strategy (per core):
  - Inputs are transposed on-chip (PE transpose) to X^T [d, s] so every GEMM
    keeps its contraction dim on SBUF partitions.
  - Q, K are produced transposed (Q^T/K^T: [feature, s]); V natural [s, feature].
  - Scores are computed transposed (scores^T [k, q]) two heads at a time using
    PE row-tiling (each head only uses 64 contraction rows).
  - exp(x/8) on ScalarE straight out of PSUM; no max-subtraction (scores are
    small by construction, fp32 exp is safe).
  - ctx^T [d, q] = V_ext^T-style matmul with an appended ones column producing
    the softmax denominator for free; normalization via a broadcast reciprocal.
  - ctx^T feeds the output projection as the stationary operand, producing the
    output in natural [s, d] layout for gelu/residual/LayerNorm.
"""

import numpy as np

S, D, H, DH = 1024, 1024, 16, 64
EPS = 1e-5
NCORES = 8
P = 128
SC = S // P    # seq chunks (8)
DC = D // P    # feature chunks (8)
HP = H // 2    # head pairs (8)

_cache = {}


def _build(flags, debug=False):
    from contextlib import ExitStack

    import concourse.bass as bass
    import concourse.mybir as mybir
    import concourse.tile as tile
    from concourse import bacc
    from concourse.masks import make_identity

    f32 = mybir.dt.float32
    f32r = mybir.dt.float32r
    bf16 = mybir.dt.bfloat16
    AF = mybir.ActivationFunctionType
    Alu = mybir.AluOpType

    use_bq, use_bk, use_bv, use_bo, use_gam, use_bet = flags

    nc = bacc.Bacc(None, target_bir_lowering=False)

    xq = nc.dram_tensor("xq", [S, D], f32, kind="ExternalInput")
    xk = nc.dram_tensor("xk", [S, D], f32, kind="ExternalInput")
    xv = nc.dram_tensor("xv", [S, D], f32, kind="ExternalInput")
    wq = nc.dram_tensor("wq", [D, D], f32, kind="ExternalInput")
    wk = nc.dram_tensor("wk", [D, D], f32, kind="ExternalInput")
    wv = nc.dram_tensor("wv", [D, D], f32, kind="ExternalInput")
    wo = nc.dram_tensor("wo", [D, D], f32, kind="ExternalInput")
    bq = nc.dram_tensor("bq", [D], f32, kind="ExternalInput")
    bk = nc.dram_tensor("bk", [D], f32, kind="ExternalInput")
    bv = nc.dram_tensor("bv", [D], f32, kind="ExternalInput")
    bo = nc.dram_tensor("bo", [D], f32, kind="ExternalInput")
    gam = nc.dram_tensor("gam", [D], f32, kind="ExternalInput")
    bet = nc.dram_tensor("bet", [D], f32, kind="ExternalInput")
    out = nc.dram_tensor("out", [S, D], f32, kind="ExternalOutput")
    if debug:
        d_qt = nc.dram_tensor("d_qt", [P, DC, S], f32, kind="ExternalOutput")
        d_kt = nc.dram_tensor("d_kt", [P, DC, S], f32, kind="ExternalOutput")
        d_vx = nc.dram_tensor("d_vx", [P, SC, H, DH + 1], f32, kind="ExternalOutput")
        d_es = nc.dram_tensor("d_es", [P, 1024], f32, kind="ExternalOutput")
        d_pc = nc.dram_tensor("d_pc", [DH + 1, 512], f32, kind="ExternalOutput")
        d_ct = nc.dram_tensor("d_ct", [P, DC, S], f32, kind="ExternalOutput")

    def r32(ap):
        return ap.bitcast(f32r)

    with tile.TileContext(nc) as tc, ExitStack() as top:
        consts = top.enter_context(tc.tile_pool(name="consts", bufs=1))
        bigp = top.enter_context(tc.tile_pool(name="bigp", bufs=1))
        wp = top.enter_context(tc.tile_pool(name="wp", bufs=1))

        ident = consts.tile([P, P], f32, tag="ident")
        make_identity(nc, ident[:])

        need_ones = use_bv or use_bo
        if need_ones:
            ones1 = consts.tile([1, P], f32r, tag="ones1")
            nc.vector.memset(ones1[:], 1.0)
        if use_bq:
            bq_sb = consts.tile([P, DC], f32, tag="bq")
            nc.sync.dma_start(out=bq_sb[:], in_=bq[:].rearrange("(c p) -> p c", p=P))
        if use_bk:
            bk_sb = consts.tile([P, DC], f32, tag="bk")
            nc.sync.dma_start(out=bk_sb[:], in_=bk[:].rearrange("(c p) -> p c", p=P))
        if use_bv:
            bv_sb = consts.tile([1, D], f32r, tag="bv")
            nc.sync.dma_start(out=bv_sb[:], in_=bv[:].rearrange("d -> 1 d").bitcast(f32r))
        if use_bo:
            bo_sb = consts.tile([1, D], f32r, tag="bo")
            nc.sync.dma_start(out=bo_sb[:], in_=bo[:].rearrange("d -> 1 d").bitcast(f32r))
        if use_gam:
            gam_bc = consts.tile([P, D], f32, tag="gam")
            nc.sync.dma_start(
                out=gam_bc[:],
                in_=bass.AP(tensor=gam[:].tensor, offset=0, ap=[[0, P], [1, D]]),
            )
        if use_bet:
            bet_bc = consts.tile([P, D], f32, tag="bet")
            nc.sync.dma_start(
                out=bet_bc[:],
                in_=bass.AP(tensor=bet[:].tensor, offset=0, ap=[[0, P], [1, D]]),
            )
        eps_sb = consts.tile([P, 1], f32, tag="eps")
        nc.vector.memset(eps_sb[:], EPS)

        def load_w(wd):
            w_sb = wp.tile([P, DC, D], f32r, tag="w")
            for kc in range(DC):
                nc.sync.dma_start(out=w_sb[:, kc, :], in_=wd[kc * P:(kc + 1) * P, :].bitcast(f32r))
            return w_sb

        with tc.tile_pool(name="qkvp", bufs=1) as qkvp:
            qt = qkvp.tile([P, DC, S], f32r, tag="qt")
            kt = qkvp.tile([P, DC, S], f32r, tag="kt")
            vx = qkvp.tile([P, SC, H, DH + 1], bf16, tag="vx")
            ones16 = consts.tile([P, H], f32, tag="ones16")
            nc.vector.memset(ones16[:], 1.0)
            for sc in range(SC):
                nc.vector.tensor_copy(vx[:, sc, :, DH], ones16[:])

            # ---------------- Phase A: transposes + projections ----------------
            with tc.tile_pool(name="xnp", bufs=4) as xnp, \
                 tc.tile_pool(name="tp_ps", bufs=4, space="PSUM") as tp_ps, \
                 tc.tile_pool(name="pj_ps", bufs=4, space="PSUM") as pj_ps:

                def transpose_in(xd):
                    xt = bigp.tile([P, DC, S], f32r, tag="big")
                    for sc in range(SC):
                        xn = xnp.tile([P, D], f32, tag="xn")
                        nc.sync.dma_start(out=xn[:], in_=xd[sc * P:(sc + 1) * P, :])
                        for kc in range(DC):
                            pt = tp_ps.tile([P, P], f32, tag="tp")
                            nc.tensor.transpose(
                                pt[:], xn[:, kc * P:(kc + 1) * P], ident[:]
                            )
                            dst_blk = xt[:, kc, sc * P:(sc + 1) * P]
                            if kc % 2 == 0:
                                nc.vector.tensor_copy(dst_blk, pt[:])
                            else:
                                nc.scalar.copy(dst_blk, pt[:])
                    return xt

                def project_T(xt, w_sb, dst, bias_sb):
                    # dst[p, mc, s] = (X @ W)[s, mc*128+p] (+ bias)
                    # two accumulation groups interleaved so consecutive PE
                    # matmuls target different PSUM banks (pipelines better
                    # than an 8-deep same-bank chain)
                    for sh in range(2):
                        ssl = slice(sh * 512, (sh + 1) * 512)
                        for mc0 in range(0, DC, 2):
                            psA = pj_ps.tile([P, 512], f32, tag="pj", name="psA")
                            psB = pj_ps.tile([P, 512], f32, tag="pj", name="psB")
                            for kc in range(DC):
                                for ps, mc in ((psA, mc0), (psB, mc0 + 1)):
                                    nc.tensor.matmul(
                                        ps[:],
                                        r32(w_sb[:, kc, mc * P:(mc + 1) * P]),
                                        r32(xt[:, kc, ssl]),
                                        start=(kc == 0),
                                        stop=(kc == DC - 1),
                                    )
                            for i, (ps, mc) in enumerate(((psA, mc0), (psB, mc0 + 1))):
                                d = dst[:, mc, ssl]
                                if bias_sb is not None:
                                    nc.vector.tensor_scalar_add(
                                        d, in0=ps[:], scalar1=bias_sb[:, mc:mc + 1]
                                    )
                                elif i == 0:
                                    nc.vector.tensor_copy(d, ps[:])
                                else:
                                    nc.scalar.copy(d, ps[:])

                def project_V(xt, w_sb):
                    # vx[p, sc, h, d] = (Xv @ Wv)[sc*128+p, h*64+d] (+ bias)
                    for sc in range(SC):
                        psA = pj_ps.tile([P, 512], f32, tag="pj", name="psA")
                        psB = pj_ps.tile([P, 512], f32, tag="pj", name="psB")
                        for kc in range(DC):
                            for ps, nh in ((psA, 0), (psB, 1)):
                                nc.tensor.matmul(
                                    ps[:],
                                    r32(xt[:, kc, sc * P:(sc + 1) * P]),
                                    r32(w_sb[:, kc, nh * 512:(nh + 1) * 512]),
                                    start=(kc == 0),
                                    stop=(kc == DC - 1) and not use_bv,
                                )
                        if use_bv:
                            for ps, nh in ((psA, 0), (psB, 1)):
                                nc.tensor.matmul(
                                    ps[:],
                                    ones1[:],
                                    r32(bv_sb[0:1, nh * 512:(nh + 1) * 512]),
                                    start=False,
                                    stop=True,
                                )
                        for i, (ps, nh) in enumerate(((psA, 0), (psB, 1))):
                            dst = vx[:, sc, nh * 8:(nh + 1) * 8, 0:DH]
                            srcp = ps[:].rearrange("p (h d) -> p h d", d=DH)
                            if i == 0:
                                nc.vector.tensor_copy(dst, srcp)
                            else:
                                nc.scalar.copy(dst, srcp)

                with nc.named_scope("proj_k"):
                    xtk = transpose_in(xk)
                    w_sb = load_w(wk)
                    project_T(xtk, w_sb, kt, bk_sb if use_bk else None)
                with nc.named_scope("proj_v"):
                    xtv = transpose_in(xv)
                    w_sb = load_w(wv)
                    project_V(xtv, w_sb)
                with nc.named_scope("proj_q"):
                    xtq = transpose_in(xq)
                    w_sb = load_w(wq)
                    project_T(xtq, w_sb, qt, bq_sb if use_bq else None)

            if debug:
                nc.sync.dma_start(out=d_qt[:], in_=qt[:].bitcast(f32))
                nc.sync.dma_start(out=d_kt[:], in_=kt[:].bitcast(f32))
                nc.sync.dma_start(out=d_vx[:], in_=vx[:].bitcast(f32))

            # ---------------- Phase B: attention ----------------
            ct = bigp.tile([P, DC, S], f32r, tag="big")
            # prefetch Wo during attention on the SWDGE queue (keeps the
            # HWDGE queue free for the latency-critical denom DMAs)
            wo_pref = wp.tile([P, DC, D], f32r, tag="w", name="wo_pref")
            wo_sb_holder = [wo_pref]
            for kc in range(DC):
                nc.gpsimd.dma_start(
                    out=wo_sb_holder[0][:, kc, :],
                    in_=wo[kc * P:(kc + 1) * P, :].bitcast(f32r),
                )
            with tc.tile_pool(name="esp", bufs=6) as esp, \
                 tc.tile_pool(name="rcp", bufs=2) as rcp, \
                 tc.tile_pool(name="tmp", bufs=2) as tmpp, \
                 tc.tile_pool(name="ps_s", bufs=2, space="PSUM") as ps_s, \
                 tc.tile_pool(name="ps_c", bufs=4, space="PSUM") as ps_c, \
                 nc.named_scope("attention"):
                for hp_i in range(HP):
                    hA, hB = 2 * hp_i, 2 * hp_i + 1
                    for qh in range(2):
                        qsl = slice(qh * 512, (qh + 1) * 512)
                        pcA = ps_c.tile([DH + 1, 512], f32, tag="pc")
                        pcB = ps_c.tile([DH + 1, 512], f32, tag="pc")
                        es_tiles = [None] * SC

                        def emit_scores(kt_i):
                            ks = slice(kt_i * P, (kt_i + 1) * P)
                            ps = ps_s.tile([P, 1024], f32, tag="ps")
                            nc.tensor.matmul(
                                ps[:, 0:512],
                                kt[0:64, hp_i, ks],
                                qt[0:64, hp_i, qsl],
                                start=True, stop=True,
                                tile_position=(0, 0),
                            )
                            nc.tensor.matmul(
                                ps[:, 512:1024],
                                kt[64:128, hp_i, ks],
                                qt[64:128, hp_i, qsl],
                                start=True, stop=True,
                                tile_position=(64, 0),
                            )
                            es = esp.tile([P, 1024], bf16, tag="es")
                            nc.scalar.activation(es[:], ps[:], AF.Exp, scale=0.125)
                            es_tiles[kt_i] = es

                        def emit_uctx(kt_i):
                            es = es_tiles[kt_i]
                            nc.tensor.matmul(
                                pcA[:],
                                vx[:, kt_i, hA, :],
                                es[:, 0:512],
                                start=(kt_i == 0), stop=(kt_i == SC - 1),
                            )
                            nc.tensor.matmul(
                                pcB[:],
                                vx[:, kt_i, hB, :],
                                es[:, 512:1024],
                                start=(kt_i == 0), stop=(kt_i == SC - 1),
                            )

                        # software-pipelined: scores run 2 k-tiles ahead of uctx
                        for kt_i in range(SC + 2):
                            if kt_i < SC:
                                emit_scores(kt_i)
                            if kt_i >= 2:
                                emit_uctx(kt_i - 2)

                        if debug and hp_i == 0 and qh == 0:
                            nc.sync.dma_start(out=d_es[:], in_=es_tiles[0][:].bitcast(f32))
                            dpc = tmpp.tile([DH + 1, 512], f32, tag="dpc")
                            nc.vector.tensor_copy(dpc[:], pcA[:])
                            nc.sync.dma_start(out=d_pc[:], in_=dpc[:])

                        # normalize: ctx^T = uctx^T * (1/denom) broadcast
                        for h, pc in ((hA, pcA), (hB, pcB)):
                            # recip on the denom row (partition-aligned with
                            # PSUM row 64), DMA it to partition 0 (HW
                            # partition_broadcast only reads partition 0),
                            # then broadcast to 64 partitions
                            dn = rcp.tile([DH + 1, 512], f32, tag="dn")
                            nc.vector.tensor_copy(dn[DH:DH + 1, :], pc[DH:DH + 1, :])
                            dn0 = rcp.tile([1, 512], f32, tag="dn0")
                            nc.sync.dma_start(out=dn0[:], in_=dn[DH:DH + 1, :])
                            rbc = rcp.tile([DH, 512], f32, tag="rbc")
                            nc.gpsimd.partition_broadcast(rbc[:], dn0[:])
                            # fast recip on SBUF (PSUM reads break its
                            # bitwise seed); in-place on the broadcast tile
                            nc.vector.reciprocal_approx_fast(
                                out=rbc[:], in_=rbc[:]
                            )
                            if h % 2 == 0:
                                nc.vector.tensor_mul(
                                    ct[0:64, hp_i, qsl], pc[0:DH, :], rbc[:]
                                )
                            else:
                                tmp = tmpp.tile([DH, 512], f32r, tag="tmp")
                                nc.vector.tensor_mul(tmp[:], pc[0:DH, :], rbc[:])
                                nc.sync.dma_start(
                                    out=ct[64:128, hp_i, qsl], in_=tmp[:]
                                )

        # ---------------- Phase C: output projection + gelu + residual + LN ----
        with tc.tile_pool(name="yp", bufs=SC) as yp, \
             tc.tile_pool(name="xn2", bufs=3) as xn2, \
             tc.tile_pool(name="stp", bufs=2) as stp, \
             tc.tile_pool(name="mvp", bufs=1) as mvp, \
             tc.tile_pool(name="ps_o", bufs=6, space="PSUM") as ps_o, \
             nc.named_scope("out_proj"):
            if debug:
                nc.sync.dma_start(out=d_ct[:], in_=ct[:].bitcast(f32))
            wo_sb = wo_sb_holder[0]
            mv_all = mvp.tile([P, SC, 2], f32, tag="mv")
            rstd = mvp.tile([P, SC], f32, tag="rstd")
            y_tiles = []

            def emit_chunk(sc):
                ssl = slice(sc * P, (sc + 1) * P)
                xn = xn2.tile([P, D], f32, tag="xn2")
                nc.sync.dma_start(out=xn[:], in_=xq[ssl, :])
                y = yp.tile([P, D], f32, tag="y")
                y_tiles.append(y)
                poA = ps_o.tile([P, 512], f32, tag="po", name="poA")
                poB = ps_o.tile([P, 512], f32, tag="po", name="poB")
                for mc in range(DC):
                    for po, nh in ((poA, 0), (poB, 1)):
                        nc.tensor.matmul(
                            po[:],
                            ct[:, mc, ssl],
                            wo_sb[:, mc, nh * 512:(nh + 1) * 512],
                            start=(mc == 0),
                            stop=(mc == DC - 1) and not use_bo,
                        )
                if use_bo:
                    for po, nh in ((poA, 0), (poB, 1)):
                        nc.tensor.matmul(
                            po[:],
                            ones1[:],
                            bo_sb[0:1, nh * 512:(nh + 1) * 512],
                            start=False, stop=True,
                        )
                for po, nh in ((poA, 0), (poB, 1)):
                    nsl = slice(nh * 512, (nh + 1) * 512)
                    # gelu straight from PSUM, then add residual
                    nc.scalar.activation(y[:, nsl], po[:], AF.Gelu)
                    nc.vector.tensor_add(y[:, nsl], y[:, nsl], xn[:, nsl])
                st = stp.tile([P, 2, 6], f32, tag="st")
                nc.vector.bn_stats(st[:, 0, :], y[:, 0:512])
                nc.vector.bn_stats(st[:, 1, :], y[:, 512:1024])
                nc.vector.bn_aggr(mv_all[:, sc, :], st[:])

            def emit_finalize(batch):
                # rstd for this batch of chunks, then normalize + store
                bsl = slice(batch[0], batch[-1] + 1)
                nc.scalar.activation(
                    rstd[:, bsl], mv_all[:, bsl, 1], AF.Sqrt, bias=eps_sb[:]
                )
                nc.vector.reciprocal(rstd[:, bsl], rstd[:, bsl])
                for sc in batch:
                    y = y_tiles[sc]
                    nc.vector.tensor_scalar(
                        out=y[:],
                        in0=y[:],
                        scalar1=mv_all[:, sc, 0:1],
                        scalar2=rstd[:, sc:sc + 1],
                        op0=Alu.subtract,
                        op1=Alu.mult,
                    )
                    if use_gam:
                        nc.vector.tensor_mul(y[:], y[:], gam_bc[:])
                    if use_bet:
                        nc.vector.tensor_add(y[:], y[:], bet_bc[:])
                    nc.sync.dma_start(out=out[sc * P:(sc + 1) * P, :], in_=y[:])

            for sc in range(5):
                emit_chunk(sc)
            emit_finalize(list(range(5)))
            for sc in range(5, SC):
                emit_chunk(sc)
            emit_finalize(list(range(5, SC)))

    nc.finalize()
    return nc


def _get_nc(flags):
    if flags not in _cache:
        _cache[flags] = _build(flags)
    return _cache[flags]


def kernel(q, k, v, wq, bq, wk, bk, wv, bv, wo, bo, ln_gamma, ln_beta):
    from concourse.bass_utils import run_bass_kernel_spmd

    q = np.ascontiguousarray(q, dtype=np.float32)
    k = np.ascontiguousarray(k, dtype=np.float32)
    v = np.ascontiguousarray(v, dtype=np.float32)

    flags = (
        bool(np.any(bq)), bool(np.any(bk)), bool(np.any(bv)), bool(np.any(bo)),
        not bool(np.all(ln_gamma == 1.0)), bool(np.any(ln_beta)),
    )
    nc = _get_nc(flags)

    shared = {
        "wq": np.ascontiguousarray(wq, np.float32),
        "wk": np.ascontiguousarray(wk, np.float32),
        "wv": np.ascontiguousarray(wv, np.float32),
        "wo": np.ascontiguousarray(wo, np.float32),
        "bq": np.ascontiguousarray(bq, np.float32),
        "bk": np.ascontiguousarray(bk, np.float32),
        "bv": np.ascontiguousarray(bv, np.float32),
        "bo": np.ascontiguousarray(bo, np.float32),
        "gam": np.ascontiguousarray(ln_gamma, np.float32),
        "bet": np.ascontiguousarray(ln_beta, np.float32),
    }
    in_maps = [
        {"xq": q[b], "xk": k[b], "xv": v[b], **shared} for b in range(NCORES)
    ]
    res = run_bass_kernel_spmd(nc, in_maps, core_ids=list(range(NCORES)))
    return np.stack([res.results[b]["out"] for b in range(NCORES)], axis=0)


# revision 27
# speedup vs baseline: 1.1584x; 1.0149x over previous
"""Trainium2 Bass kernel for a full MHA transformer block.

Reference computation (per batch element, data-parallel over 8 NeuronCores):
    qh/kh/vh = (x @ W + b) split into 16 heads of 64
    attn     = softmax(qh @ kh^T / sqrt(64))
    ctx      = attn @ vh
    out      = LayerNorm(gelu(ctx @ Wo + bo) + residual) * gamma + beta

Shapes: B=8, S=1024, D=1024, H=16, DH=64, fp32.

Layout strategy (per core):
  - Inputs are transposed on-chip (PE transpose) to X^T [d, s] so every GEMM
    keeps its contraction dim on SBUF partitions.
  - Q, K are produced transposed (Q^T/K^T: [feature, s]); V natural [s, feature].
  - Scores are computed transposed (scores^T [k, q]) two heads at a time using
    PE row-tiling (each head only uses 64 contraction rows).
  - exp(x/8) on ScalarE straight out of PSUM; no max-subtraction (scores are
    small by construction, fp32 exp is safe).
  - ctx^T [d, q] = V_ext^T-style matmul with an appended ones column producing
    the softmax denominator for free; normalization via a broadcast reciprocal.
  - ctx^T feeds the output projection as the stationary operand, producing the
    output in natural [s, d] layout for gelu/residual/LayerNorm.
"""

import numpy as np

S, D, H, DH = 1024, 1024, 16, 64
EPS = 1e-5
NCORES = 8
P = 128
SC = S // P    # seq chunks (8)
DC = D // P    # feature chunks (8)
HP = H // 2    # head pairs (8)

_cache = {}


def _build(flags, debug=False):
    from contextlib import ExitStack

    import concourse.bass as bass
    import concourse.mybir as mybir
    import concourse.tile as tile
    from concourse import bacc
    from concourse.masks import make_identity

    f32 = mybir.dt.float32
    f32r = mybir.dt.float32r
    bf16 = mybir.dt.bfloat16
    AF = mybir.ActivationFunctionType
    Alu = mybir.AluOpType

    use_bq, use_bk, use_bv, use_bo, use_gam, use_bet = flags

    nc = bacc.Bacc(None, target_bir_lowering=False)

    xq = nc.dram_tensor("xq", [S, D], f32, kind="ExternalInput")
    xk = nc.dram_tensor("xk", [S, D], f32, kind="ExternalInput")
    xv = nc.dram_tensor("xv", [S, D], f32, kind="ExternalInput")
    wq = nc.dram_tensor("wq", [D, D], f32, kind="ExternalInput")
    wk = nc.dram_tensor("wk", [D, D], f32, kind="ExternalInput")
    wv = nc.dram_tensor("wv", [D, D], f32, kind="ExternalInput")
    wo = nc.dram_tensor("wo", [D, D], f32, kind="ExternalInput")
    bq = nc.dram_tensor("bq", [D], f32, kind="ExternalInput")
    bk = nc.dram_tensor("bk", [D], f32, kind="ExternalInput")
    bv = nc.dram_tensor("bv", [D], f32, kind="ExternalInput")
    bo = nc.dram_tensor("bo", [D], f32, kind="ExternalInput")
    gam = nc.dram_tensor("gam", [D], f32, kind="ExternalInput")
    bet = nc.dram_tensor("bet", [D], f32, kind="ExternalInput")
    out = nc.dram_tensor("out", [S, D], f32, kind="ExternalOutput")
    if debug:
        d_qt = nc.dram_tensor("d_qt", [P, DC, S], f32, kind="ExternalOutput")
        d_kt = nc.dram_tensor("d_kt", [P, DC, S], f32, kind="ExternalOutput")
        d_vx = nc.dram_tensor("d_vx", [P, SC, H, DH + 1], f32, kind="ExternalOutput")
        d_es = nc.dram_tensor("d_es", [P, 1024], f32, kind="ExternalOutput")
        d_pc = nc.dram_tensor("d_pc", [DH + 1, 512], f32, kind="ExternalOutput")
        d_ct = nc.dram_tensor("d_ct", [P, DC, S], f32, kind="ExternalOutput")

    def r32(ap):
        return ap.bitcast(f32r)

    with tile.TileContext(nc) as tc, ExitStack() as top:
        consts = top.enter_context(tc.tile_pool(name="consts", bufs=1))
        bigp = top.enter_context(tc.tile_pool(name="bigp", bufs=1))
        wp = top.enter_context(tc.tile_pool(name="wp", bufs=1))

        ident = consts.tile([P, P], f32, tag="ident")
        make_identity(nc, ident[:])

        need_ones = use_bv or use_bo
        if need_ones:
            ones1 = consts.tile([1, P], f32r, tag="ones1")
            nc.vector.memset(ones1[:], 1.0)
        if use_bq:
            bq_sb = consts.tile([P, DC], f32, tag="bq")
            nc.sync.dma_start(out=bq_sb[:], in_=bq[:].rearrange("(c p) -> p c", p=P))
        if use_bk:
            bk_sb = consts.tile([P, DC], f32, tag="bk")
            nc.sync.dma_start(out=bk_sb[:], in_=bk[:].rearrange("(c p) -> p c", p=P))
        if use_bv:
            bv_sb = consts.tile([1, D], f32r, tag="bv")
            nc.sync.dma_start(out=bv_sb[:], in_=bv[:].rearrange("d -> 1 d").bitcast(f32r))
        if use_bo:
            bo_sb = consts.tile([1, D], f32r, tag="bo")
            nc.sync.dma_start(out=bo_sb[:], in_=bo[:].rearrange("d -> 1 d").bitcast(f32r))
        if use_gam:
            gam_bc = consts.tile([P, D], f32, tag="gam")
            nc.sync.dma_start(
                out=gam_bc[:],
                in_=bass.AP(tensor=gam[:].tensor, offset=0, ap=[[0, P], [1, D]]),
            )
        if use_bet:
            bet_bc = consts.tile([P, D], f32, tag="bet")
            nc.sync.dma_start(
                out=bet_bc[:],
                in_=bass.AP(tensor=bet[:].tensor, offset=0, ap=[[0, P], [1, D]]),
            )
        eps_sb = consts.tile([P, 1], f32, tag="eps")
        nc.vector.memset(eps_sb[:], EPS)

        def load_w(wd):
            w_sb = wp.tile([P, DC, D], f32r, tag="w")
            for kc in range(DC):
                nc.sync.dma_start(out=w_sb[:, kc, :], in_=wd[kc * P:(kc + 1) * P, :].bitcast(f32r))
            return w_sb

        with tc.tile_pool(name="qkvp", bufs=1) as qkvp:
            qt = qkvp.tile([P, DC, S], f32r, tag="qt")
            kt = qkvp.tile([P, DC, S], f32r, tag="kt")
            vx = qkvp.tile([P, SC, H, DH + 1], bf16, tag="vx")
            ones16 = consts.tile([P, H], f32, tag="ones16")
            nc.vector.memset(ones16[:], 1.0)
            for sc in range(SC):
                nc.vector.tensor_copy(vx[:, sc, :, DH], ones16[:])

            # ---------------- Phase A: transposes + projections ----------------
            with tc.tile_pool(name="xnp", bufs=5) as xnp, \
                 tc.tile_pool(name="tp_ps", bufs=4, space="PSUM") as tp_ps, \
                 tc.tile_pool(name="pj_ps", bufs=4, space="PSUM") as pj_ps:

                def transpose_in(xd):
                    xt = bigp.tile([P, DC, S], f32r, tag="big")
                    for sc in range(SC):
                        xn = xnp.tile([P, D], f32, tag="xn")
                        nc.sync.dma_start(out=xn[:], in_=xd[sc * P:(sc + 1) * P, :])
                        for kc in range(DC):
                            pt = tp_ps.tile([P, P], f32, tag="tp")
                            nc.tensor.transpose(
                                pt[:], xn[:, kc * P:(kc + 1) * P], ident[:]
                            )
                            dst_blk = xt[:, kc, sc * P:(sc + 1) * P]
                            if kc % 2 == 0:
                                nc.vector.tensor_copy(dst_blk, pt[:])
                            else:
                                nc.scalar.copy(dst_blk, pt[:])
                    return xt

                def project_T(xt, w_sb, dst, bias_sb):
                    # dst[p, mc, s] = (X @ W)[s, mc*128+p] (+ bias)
                    # two accumulation groups interleaved so consecutive PE
                    # matmuls target different PSUM banks (pipelines better
                    # than an 8-deep same-bank chain)
                    for sh in range(2):
                        ssl = slice(sh * 512, (sh + 1) * 512)
                        for mc0 in range(0, DC, 2):
                            psA = pj_ps.tile([P, 512], f32, tag="pj", name="psA")
                            psB = pj_ps.tile([P, 512], f32, tag="pj", name="psB")
                            for kc in range(DC):
                                for ps, mc in ((psA, mc0), (psB, mc0 + 1)):
                                    nc.tensor.matmul(
                                        ps[:],
                                        r32(w_sb[:, kc, mc * P:(mc + 1) * P]),
                                        r32(xt[:, kc, ssl]),
                                        start=(kc == 0),
                                        stop=(kc == DC - 1),
                                    )
                            for i, (ps, mc) in enumerate(((psA, mc0), (psB, mc0 + 1))):
                                d = dst[:, mc, ssl]
                                if bias_sb is not None:
                                    nc.vector.tensor_scalar_add(
                                        d, in0=ps[:], scalar1=bias_sb[:, mc:mc + 1]
                                    )
                                elif i == 0:
                                    nc.vector.tensor_copy(d, ps[:])
                                else:
                                    nc.scalar.copy(d, ps[:])

                def project_V(xt, w_sb):
                    # vx[p, sc, h, d] = (Xv @ Wv)[sc*128+p, h*64+d] (+ bias)
                    for sc in range(SC):
                        psA = pj_ps.tile([P, 512], f32, tag="pj", name="psA")
                        psB = pj_ps.tile([P, 512], f32, tag="pj", name="psB")
                        for kc in range(DC):
                            for ps, nh in ((psA, 0), (psB, 1)):
                                nc.tensor.matmul(
                                    ps[:],
                                    r32(xt[:, kc, sc * P:(sc + 1) * P]),
                                    r32(w_sb[:, kc, nh * 512:(nh + 1) * 512]),
                                    start=(kc == 0),
                                    stop=(kc == DC - 1) and not use_bv,
                                )
                        if use_bv:
                            for ps, nh in ((psA, 0), (psB, 1)):
                                nc.tensor.matmul(
                                    ps[:],
                                    ones1[:],
                                    r32(bv_sb[0:1, nh * 512:(nh + 1) * 512]),
                                    start=False,
                                    stop=True,
                                )
                        for i, (ps, nh) in enumerate(((psA, 0), (psB, 1))):
                            dst = vx[:, sc, nh * 8:(nh + 1) * 8, 0:DH]
                            srcp = ps[:].rearrange("p (h d) -> p h d", d=DH)
                            if i == 0:
                                nc.vector.tensor_copy(dst, srcp)
                            else:
                                nc.scalar.copy(dst, srcp)

                with nc.named_scope("proj_k"):
                    xtk = transpose_in(xk)
                    w_sb = load_w(wk)
                    project_T(xtk, w_sb, kt, bk_sb if use_bk else None)
                with nc.named_scope("proj_v"):
                    xtv = transpose_in(xv)
                    w_sb = load_w(wv)
                    project_V(xtv, w_sb)
                with nc.named_scope("proj_q"):
                    xtq = transpose_in(xq)
                    w_sb = load_w(wq)
                    project_T(xtq, w_sb, qt, bq_sb if use_bq else None)

            if debug:
                nc.sync.dma_start(out=d_qt[:], in_=qt[:].bitcast(f32))
                nc.sync.dma_start(out=d_kt[:], in_=kt[:].bitcast(f32))
                nc.sync.dma_start(out=d_vx[:], in_=vx[:].bitcast(f32))

            # ---------------- Phase B: attention ----------------
            ct = bigp.tile([P, DC, S], f32r, tag="big")
            # prefetch Wo during attention on the SWDGE queue (keeps the
            # HWDGE queue free for the latency-critical denom DMAs)
            wo_pref = wp.tile([P, DC, D], f32r, tag="w", name="wo_pref")
            wo_sb_holder = [wo_pref]
            for kc in range(DC):
                nc.gpsimd.dma_start(
                    out=wo_sb_holder[0][:, kc, :],
                    in_=wo[kc * P:(kc + 1) * P, :].bitcast(f32r),
                )
            with tc.tile_pool(name="esp", bufs=6) as esp, \
                 tc.tile_pool(name="rcp", bufs=2) as rcp, \
                 tc.tile_pool(name="tmp", bufs=2) as tmpp, \
                 tc.tile_pool(name="ps_s", bufs=2, space="PSUM") as ps_s, \
                 tc.tile_pool(name="ps_c", bufs=4, space="PSUM") as ps_c, \
                 nc.named_scope("attention"):
                for hp_i in range(HP):
                    hA, hB = 2 * hp_i, 2 * hp_i + 1
                    for qh in range(2):
                        qsl = slice(qh * 512, (qh + 1) * 512)
                        pcA = ps_c.tile([DH + 1, 512], f32, tag="pc")
                        pcB = ps_c.tile([DH + 1, 512], f32, tag="pc")
                        es_tiles = [None] * SC

                        def emit_scores(kt_i):
                            ks = slice(kt_i * P, (kt_i + 1) * P)
                            ps = ps_s.tile([P, 1024], f32, tag="ps")
                            nc.tensor.matmul(
                                ps[:, 0:512],
                                kt[0:64, hp_i, ks],
                                qt[0:64, hp_i, qsl],
                                start=True, stop=True,
                                tile_position=(0, 0),
                            )
                            nc.tensor.matmul(
                                ps[:, 512:1024],
                                kt[64:128, hp_i, ks],
                                qt[64:128, hp_i, qsl],
                                start=True, stop=True,
                                tile_position=(64, 0),
                            )
                            es = esp.tile([P, 1024], bf16, tag="es")
                            nc.scalar.activation(es[:], ps[:], AF.Exp, scale=0.125)
                            es_tiles[kt_i] = es

                        def emit_uctx(kt_i):
                            es = es_tiles[kt_i]
                            nc.tensor.matmul(
                                pcA[:],
                                vx[:, kt_i, hA, :],
                                es[:, 0:512],
                                start=(kt_i == 0), stop=(kt_i == SC - 1),
                            )
                            nc.tensor.matmul(
                                pcB[:],
                                vx[:, kt_i, hB, :],
                                es[:, 512:1024],
                                start=(kt_i == 0), stop=(kt_i == SC - 1),
                            )

                        # software-pipelined: scores run 2 k-tiles ahead of uctx
                        for kt_i in range(SC + 2):
                            if kt_i < SC:
                                emit_scores(kt_i)
                            if kt_i >= 2:
                                emit_uctx(kt_i - 2)

                        if debug and hp_i == 0 and qh == 0:
                            nc.sync.dma_start(out=d_es[:], in_=es_tiles[0][:].bitcast(f32))
                            dpc = tmpp.tile([DH + 1, 512], f32, tag="dpc")
                            nc.vector.tensor_copy(dpc[:], pcA[:])
                            nc.sync.dma_start(out=d_pc[:], in_=dpc[:])

                        # normalize: ctx^T = uctx^T * (1/denom) broadcast
                        for h, pc in ((hA, pcA), (hB, pcB)):
                            # recip on the denom row (partition-aligned with
                            # PSUM row 64), DMA it to partition 0 (HW
                            # partition_broadcast only reads partition 0),
                            # then broadcast to 64 partitions
                            dn = rcp.tile([DH + 1, 512], f32, tag="dn")
                            nc.vector.tensor_copy(dn[DH:DH + 1, :], pc[DH:DH + 1, :])
                            dn0 = rcp.tile([1, 512], f32, tag="dn0")
                            nc.sync.dma_start(out=dn0[:], in_=dn[DH:DH + 1, :])
                            rbc = rcp.tile([DH, 512], f32, tag="rbc")
                            nc.gpsimd.partition_broadcast(rbc[:], dn0[:])
                            # fast recip on SBUF (PSUM reads break its
                            # bitwise seed); in-place on the broadcast tile
                            nc.vector.reciprocal_approx_fast(
                                out=rbc[:], in_=rbc[:]
                            )
                            if h % 2 == 0:
                                nc.vector.tensor_mul(
                                    ct[0:64, hp_i, qsl], pc[0:DH, :], rbc[:]
                                )
                            else:
                                tmp = tmpp.tile([DH, 512], f32r, tag="tmp")
                                nc.vector.tensor_mul(tmp[:], pc[0:DH, :], rbc[:])
                                nc.sync.dma_start(
                                    out=ct[64:128, hp_i, qsl], in_=tmp[:]
                                )

        # ---------------- Phase C: output projection + gelu + residual + LN ----
        with tc.tile_pool(name="yp", bufs=SC) as yp, \
             tc.tile_pool(name="xn2", bufs=4) as xn2, \
             tc.tile_pool(name="stp", bufs=4) as stp, \
             tc.tile_pool(name="mvp", bufs=1) as mvp, \
             tc.tile_pool(name="ps_o", bufs=6, space="PSUM") as ps_o, \
             nc.named_scope("out_proj"):
            if debug:
                nc.sync.dma_start(out=d_ct[:], in_=ct[:].bitcast(f32))
            wo_sb = wo_sb_holder[0]
            mv_all = mvp.tile([P, SC, 2], f32, tag="mv")
            rstd = mvp.tile([P, SC], f32, tag="rstd")
            y_tiles = []

            def emit_chunk(sc):
                ssl = slice(sc * P, (sc + 1) * P)
                xn = xn2.tile([P, D], f32, tag="xn2")
                nc.sync.dma_start(out=xn[:], in_=xq[ssl, :])
                y = yp.tile([P, D], f32, tag="y")
                y_tiles.append(y)
                poA = ps_o.tile([P, 512], f32, tag="po", name="poA")
                poB = ps_o.tile([P, 512], f32, tag="po", name="poB")
                for mc in range(DC):
                    for po, nh in ((poA, 0), (poB, 1)):
                        nc.tensor.matmul(
                            po[:],
                            ct[:, mc, ssl],
                            wo_sb[:, mc, nh * 512:(nh + 1) * 512],
                            start=(mc == 0),
                            stop=(mc == DC - 1) and not use_bo,
                        )
                if use_bo:
                    for po, nh in ((poA, 0), (poB, 1)):
                        nc.tensor.matmul(
                            po[:],
                            ones1[:],
                            bo_sb[0:1, nh * 512:(nh + 1) * 512],
                            start=False, stop=True,
                        )
                for po, nh in ((poA, 0), (poB, 1)):
                    nsl = slice(nh * 512, (nh + 1) * 512)
                    # gelu straight from PSUM, then add residual
                    nc.scalar.activation(y[:, nsl], po[:], AF.Gelu)
                    nc.vector.tensor_add(y[:, nsl], y[:, nsl], xn[:, nsl])
                st = stp.tile([P, 2, 6], f32, tag="st")
                nc.vector.bn_stats(st[:, 0, :], y[:, 0:512])
                nc.vector.bn_stats(st[:, 1, :], y[:, 512:1024])
                nc.vector.bn_aggr(mv_all[:, sc, :], st[:])

            def emit_finalize(batch):
                # rstd for this batch of chunks, then normalize + store
                bsl = slice(batch[0], batch[-1] + 1)
                nc.scalar.activation(
                    rstd[:, bsl], mv_all[:, bsl, 1], AF.Sqrt, bias=eps_sb[:]
                )
                nc.vector.reciprocal(rstd[:, bsl], rstd[:, bsl])
                for sc in batch:
                    y = y_tiles[sc]
                    nc.vector.tensor_scalar(
                        out=y[:],
                        in0=y[:],
                        scalar1=mv_all[:, sc, 0:1],
                        scalar2=rstd[:, sc:sc + 1],
                        op0=Alu.subtract,
                        op1=Alu.mult,
                    )
                    if use_gam:
                        nc.vector.tensor_mul(y[:], y[:], gam_bc[:])
                    if use_bet:
                        nc.vector.tensor_add(y[:], y[:], bet_bc[:])
                    nc.sync.dma_start(out=out[sc * P:(sc + 1) * P, :], in_=y[:])

            for sc in range(5):
                emit_chunk(sc)
            emit_finalize(list(range(5)))
            for sc in range(5, SC):
                emit_chunk(sc)
            emit_finalize(list(range(5, SC)))

    nc.finalize()
    return nc


def _get_nc(flags):
    if flags not in _cache:
        _cache[flags] = _build(flags)
    return _cache[flags]


def kernel(q, k, v, wq, bq, wk, bk, wv, bv, wo, bo, ln_gamma, ln_beta):
    from concourse.bass_utils import run_bass_kernel_spmd

    q = np.ascontiguousarray(q, dtype=np.float32)
    k = np.ascontiguousarray(k, dtype=np.float32)
    v = np.ascontiguousarray(v, dtype=np.float32)

    flags = (
        bool(np.any(bq)), bool(np.any(bk)), bool(np.any(bv)), bool(np.any(bo)),
        not bool(np.all(ln_gamma == 1.0)), bool(np.any(ln_beta)),
    )
    nc = _get_nc(flags)

    shared = {
        "wq": np.ascontiguousarray(wq, np.float32),
        "wk": np.ascontiguousarray(wk, np.float32),
        "wv": np.ascontiguousarray(wv, np.float32),
        "wo": np.ascontiguousarray(wo, np.float32),
        "bq": np.ascontiguousarray(bq, np.float32),
        "bk": np.ascontiguousarray(bk, np.float32),
        "bv": np.ascontiguousarray(bv, np.float32),
        "bo": np.ascontiguousarray(bo, np.float32),
        "gam": np.ascontiguousarray(ln_gamma, np.float32),
        "bet": np.ascontiguousarray(ln_beta, np.float32),
    }
    in_maps = [
        {"xq": q[b], "xk": k[b], "xv": v[b], **shared} for b in range(NCORES)
    ]
    res = run_bass_kernel_spmd(nc, in_maps, core_ids=list(range(NCORES)))
    return np.stack([res.results[b]["out"] for b in range(NCORES)], axis=0)
